# revision 5
# baseline (speedup 1.0000x reference)
"""DCRNN (nn_DCRNNModel_nextTimePred) Bass/Tile kernel for 8 TRN2 NeuronCores, v2.

Sharding: data-parallel over batch (B=256 -> 32 per core), weights replicated.
Layout: activations feature-major [feat<=128, R] with R = 32*19 = 608 rows
zero-padded to 684 = 6 tiles x 114 (6 batch x 19 nodes per tile).

v2 structural changes vs v1:
- fp16 state + fp16 elementwise GRU tail (DVE 2x mode), no separate casts.
- Each layer's node-mix (BD-matmul of its fresh h) is computed ONCE per step
  and shared by (a) that layer's own gates next step and (b) the next layer's
  x-side this step.  m=0 term is the state tensor itself.
- Decoder projection feedback folded into the weights: (P_k(h1 Wp))Wx_k =
  (P_k h1)(Wp Wx_k), so d0's x-side consumes the already-computed mix of h1
  with combined weights WW_k = Wp @ Wx_k.  proj is computed only for output,
  off the critical path.  (bp = 0 for this problem so no bias correction.)
- Gate/cand matmuls at N=342 (full PSUM half) instead of N=114.
- State mixes use DMA transposes (cross-step latency tolerant); the rh mix and
  the decoder's h0-mix (same-step consumers) use PE transposes.  Transpose and
  mix outputs are paired two-j-tiles-per-PSUM-bank to halve copy count.
"""
import numpy as np

import concourse.bass as bass
import concourse.mybir as mybir
from concourse import bacc, tile
from concourse.bass_utils import run_bass_kernel_spmd

F16 = mybir.dt.float16
F32 = mybir.dt.float32
AF = mybir.ActivationFunctionType

NCORES = 8
B, T, N, DIN, U, O = 256, 64, 19, 100, 128, 100
BC = B // NCORES          # 32 batch per core
JT = 114                  # transpose window width (max tile rows)
J = 6                     # row tiles
R = 608                   # compact rows (no padding)
RV = BC * N               # valid rows = 608
HF = ((0, 304, 0, 3), (304, 304, 3, 6))  # half-chunks: (lo, sz, j0, j1)
RALL = 648   # transpose windows read [off, off+114) <= 646; zeros beyond 608
TOFF = (0, 114, 228, 304, 418, 532)      # compact tile offsets
TSZ = (114, 114, 76, 114, 114, 76)       # tile row counts
TB0 = (0, 6, 12, 16, 22, 28)             # first batch per tile
TNB = (6, 6, 4, 6, 6, 4)                 # batches per tile

CELLS = ("e0", "e1", "d0", "d1")
CELL_DIN = {"e0": DIN, "e1": U, "d0": DIN, "d1": U}


def _pack_layouts():
    p16, off = [], 0
    def add16(name, p, shape):
        nonlocal off
        n = int(np.prod(shape))
        p16.append((name, p, tuple(shape), off))
        off += n
    add16("bdcat3", JT, (J, 3 * JT))
    add16("bd12", JT, (J, 2 * JT))
    add16("bd12s", JT, (2, 152))
    add16("bdc3s", JT, (2, 228))
    add16("ident", 128, (128,))
    add16("wp", U, (O,))
    add16("d0_ww", U, (3, 384))
    for c in CELLS:
        add16(f"{c}_wx", CELL_DIN[c], (3, 384))
        add16(f"{c}_wgh", U, (3, 256))
        add16(f"{c}_wch", U, (3, 128))
    f16_total = off
    p32, off = [], 0
    def add32(name, p, shape):
        nonlocal off
        n = int(np.prod(shape))
        p32.append((name, p, tuple(shape), off))
        off += n
    for c in CELLS:
        add32(f"{c}_bg", U, (2,))
        add32(f"{c}_bgn", U, (1,))
        add32(f"{c}_bc", U, (1,))
    add32("bp", O, (1,))
    return p16, f16_total, p32, off


PACK16, F16TOT, PACK32, F32TOT = _pack_layouts()


# --------------------------------------------------------------------------
# host-side weight preparation
# --------------------------------------------------------------------------

def _prep_host(inputs):
    f32 = np.float32
    S = np.asarray(inputs["support"], f32)
    P = [np.eye(N, dtype=f32), S, 2.0 * (S @ S) - np.eye(N, dtype=f32)]

    def bd_t(m, nb):
        Z = np.zeros((JT, JT), f32)
        for b in range(nb):
            Z[b * N:(b + 1) * N, b * N:(b + 1) * N] = P[m].T
        return Z

    vals = {}
    bdcat3 = np.zeros((JT, J, 3 * JT), f32)
    bd12 = np.zeros((JT, J, 2 * JT), f32)
    for j in range(J):
        nb = TNB[j]
        eye = np.zeros((JT, JT), f32)
        eye[:nb * N, :nb * N] = np.eye(nb * N, dtype=f32)
        bdcat3[:, j] = np.concatenate([eye, bd_t(1, nb), bd_t(2, nb)], axis=1)
        bd12[:, j] = np.concatenate([bd_t(1, nb), bd_t(2, nb)], axis=1)
    vals["bdcat3"] = bdcat3
    vals["bd12"] = bd12
    bd12s = np.zeros((JT, 2, 152), f32)
    bdc3s = np.zeros((JT, 2, 228), f32)
    for i, j in enumerate((2, 5)):
        bd12s[:, i] = np.concatenate(
            [bd_t(1, 4)[:, :76], bd_t(2, 4)[:, :76]], axis=1)
        ey = np.zeros((JT, 76), f32)
        ey[:76] = np.eye(76, dtype=f32)
        bdc3s[:, i] = np.concatenate(
            [ey, bd_t(1, 4)[:, :76], bd_t(2, 4)[:, :76]], axis=1)
    vals["bd12s"] = bd12s
    vals["bdc3s"] = bdc3s
    vals["ident"] = np.eye(128, dtype=f32)

    for c in CELLS:
        din = CELL_DIN[c]
        Wg = np.asarray(inputs[f"{c}_Wg"], f32)   # [(din+U)*3, 2U]
        Wc = np.asarray(inputs[f"{c}_Wc"], f32)   # [(din+U)*3, U]
        wx, wgh, wch = [], [], []
        for m in range(3):
            Wg_m, Wc_m = Wg[m::3], Wc[m::3]       # [(din+U), .]
            wx.append(np.concatenate([Wg_m[:din], Wc_m[:din]], axis=1))  # [din,384]
            wgh.append(Wg_m[din:])                # [U, 256]
            wch.append(Wc_m[din:])                # [U, 128]
        vals[f"{c}_wx"] = np.stack(wx, axis=1)    # [din, 3, 384]
        vals[f"{c}_wgh"] = np.stack(wgh, axis=1)  # [U, 3, 256]
        vals[f"{c}_wch"] = np.stack(wch, axis=1)  # [U, 3, 128]
        bg = np.asarray(inputs[f"{c}_bg"], f32)
        vals[f"{c}_bg"] = np.stack([bg[:U], bg[U:]], axis=1)  # [U, 2]
        vals[f"{c}_bgn"] = -bg[U:].reshape(U, 1)              # for 1-u = sig(-z)
        vals[f"{c}_bc"] = np.asarray(inputs[f"{c}_bc"], f32).reshape(U, 1)
    Wp = np.asarray(inputs["Wp"], f32)
    vals["wp"] = Wp                               # [U, O]
    vals["bp"] = np.asarray(inputs["bp"], f32).reshape(O, 1)
    # decoder projection-feedback fold: WW_m = Wp @ wx_m(d0)  [U, 384]
    vals["d0_ww"] = np.stack([Wp @ vals["d0_wx"][:, m] for m in range(3)], axis=1)
    assert np.abs(np.asarray(inputs["bp"], f32)).max() == 0.0, \
        "nonzero bp needs the rank-3 bias correction (not emitted)"

    pack16 = np.zeros((128, F16TOT), np.float16)
    for name, p, shape, off in PACK16:
        n = int(np.prod(shape))
        pack16[:p, off:off + n] = vals[name].reshape(p, n).astype(np.float16)
    pack32 = np.zeros((128, F32TOT), f32)
    for name, p, shape, off in PACK32:
        n = int(np.prod(shape))
        pack32[:p, off:off + n] = vals[name].reshape(p, n)
    return {"wpack16": pack16, "wpack32": pack32}


def _prep_xenc(enc, core, t_enc=T):
    """per-core encoder input -> [T, J, JT, DIN] fp16, zero row padding."""
    e = np.asarray(enc[core * BC:(core + 1) * BC, :t_enc], np.float32)
    e = e.transpose(1, 0, 2, 3)                                 # [T, BC, N, DIN]
    out = np.zeros((t_enc, J, JT, DIN), np.float16)
    for j in range(J):
        out[:, j, :TNB[j] * N] = \
            e[:, TB0[j]:TB0[j] + TNB[j]].reshape(t_enc, TNB[j] * N, DIN)
    return out


# --------------------------------------------------------------------------
# program builder
# --------------------------------------------------------------------------

def build_program(t_enc=T, t_dec=T):
    nc = bacc.Bacc()
    d = {}
    d["xenc"] = nc.dram_tensor("xenc", [t_enc, J, JT, DIN], F16, kind="ExternalInput")
    d["wpack16"] = nc.dram_tensor("wpack16", [128, F16TOT], F16, kind="ExternalInput")
    d["wpack32"] = nc.dram_tensor("wpack32", [128, F32TOT], F32, kind="ExternalInput")
    d["out"] = nc.dram_tensor("out", [t_dec, O, R], F32, kind="ExternalOutput")
    with tile.TileContext(nc) as tc:
        _emit(nc, tc, d, t_enc, t_dec)
    nc.finalize()
    return nc


def _emit(nc, tc, d, t_enc, t_dec):
    import contextlib
    stack = contextlib.ExitStack()
    with stack:
        perm = stack.enter_context(tc.tile_pool(name="perm", bufs=1))
        xpool = stack.enter_context(tc.tile_pool(name="xin", bufs=3))
        xcat = stack.enter_context(tc.tile_pool(name="xcat", bufs=3))
        m0pool = stack.enter_context(tc.tile_pool(name="m0p", bufs=4))
        s12p = stack.enter_context(tc.tile_pool(name="s12", bufs=3))
        hrm = stack.enter_context(tc.tile_pool(name="hrm", bufs=6))
        pg0 = stack.enter_context(tc.tile_pool(name="pg0", bufs=2, space="PSUM"))
        pg1 = stack.enter_context(tc.tile_pool(name="pg1", bufs=1, space="PSUM"))
        pcd = stack.enter_context(tc.tile_pool(name="pcd", bufs=1, space="PSUM"))
        SCR_BUFS = 4
        scr = stack.enter_context(tc.tile_pool(name="scr", bufs=SCR_BUFS, space="PSUM"))

        # ---- persistent weights ----
        wp16 = perm.tile([128, F16TOT], F16, name="wp16", tag="wp16")
        wp32 = perm.tile([128, F32TOT], F32, name="wp32", tag="wp32")
        nc.sync.dma_start(wp16[:], d["wpack16"][:])
        nc.sync.dma_start(wp32[:], d["wpack32"][:])
        w = {}
        for name, p, shape, off in PACK16:
            n = int(np.prod(shape))
            ap = wp16[:p, off:off + n]
            if len(shape) > 1:
                ap = ap.rearrange("p (a b) -> p a b", a=shape[0])
            w[name] = ap
        for name, p, shape, off in PACK32:
            n = int(np.prod(shape))
            w[name] = wp32[:p, off:off + n]

        # ---- persistent state (all fp16) ----
        h0seq = perm.tile([U, t_enc, RALL], F16, tag="h0seq")   # h0 after step t
        h1T = perm.tile([U, RALL], F16, tag="h1T")              # h1 state (enc+dec)
        hd0 = perm.tile([U, RALL], F16, tag="hd0")              # decoder h0 state
        hz16 = perm.tile([U, RALL], F16, tag="hz16")            # zeros
        zmix = perm.tile([U, 2, R], F16, tag="zmix")            # zero mix
        m1 = perm.tile([U, 2, R], F16, tag="m1")                # mix of h1 (P1,P2)
        m0d = perm.tile([U, 2, R], F16, tag="m0d")              # decoder mix of h0
        for tl in (h0seq, h1T, hd0, hz16, zmix, m1, m0d):
            nc.gpsimd.memset(tl[:], 0.0)

        # per-layer elementwise work tiles (a: layer0 cells, b: layer1 cells)
        wk = {}
        for sfx in ("a", "b"):
            wk[sfx] = {
                k: perm.tile([U, RALL if k == "rh" else R], F16,
                             tag=f"{k}{sfx}", name=f"{k}{sfx}")
                for k in ("rT", "uT", "cT", "tmp", "rh")
            }
            nc.gpsimd.memset(wk[sfx]["rh"][:], 0.0)
        LAYER_SFX = {"e0": "a", "d0": "a", "e1": "b", "d1": "b"}

        projT32 = perm.tile([O, R], F32, tag="projT32")
        tokT = perm.tile([1, 8], F16, tag="tokT")

        # ------------------------------------------------------------------
        # one-wait-per-matmul machinery (see v1 notes): pe_fence plants tiny
        # LDWEIGHTS wait-carriers; mm ties matmuls behind the latest fence.
        # ------------------------------------------------------------------
        from concourse.tile import add_dep_helper

        cur_fence = [None]

        def _raw(i):
            return i.ins if hasattr(i, "ins") and not isinstance(i.ins, list) else i

        def pe_fence(*tok_aps):
            for tok in tok_aps:
                if tok is None:
                    continue
                i = nc.tensor.ldweights(weights=tok)
                if cur_fence[0] is not None:
                    add_dep_helper(_raw(i), _raw(cur_fence[0]), sync=False)
                cur_fence[0] = i

        def mm(*args, after=None, **kw):
            i = nc.tensor.matmul(*args, **kw)
            if cur_fence[0] is not None:
                add_dep_helper(_raw(i), _raw(cur_fence[0]), sync=False)
            if after is not None:
                add_dep_helper(_raw(i), _raw(after), sync=False)
            return i

        def copy_act(dst, src):
            return nc.scalar.activation(dst, src, AF.Copy)

        def copy_dve(dst, src):
            return nc.vector.tensor_copy(dst, src)

        scr_state = {"k": 0, "toks": {}}

        def scr_alloc(shape):
            k = scr_state["k"]
            scr_state["k"] += 1
            tok = scr_state["toks"].get(k - SCR_BUFS)
            if tok is not None:
                pe_fence(tok)
            return scr.tile(shape, F32, tag="scr", name="scr"), k

        def scr_done(k, reader_instr, tok):
            scr_state["toks"][k] = tok
            return reader_instr

        # ------------------------------------------------------------------
        # node-mix of a feature-major fp16 tensor: src [U, RALL] -> dst [U,2,R]
        # (P1, P2 diffusion terms).  tr='pe': PE transpose (low latency);
        # tr='dma': DMA-xbar transpose (no PE/copy cost, ~us latency).
        # j-tiles are paired two-per-PSUM-bank to halve copy count.
        # ------------------------------------------------------------------
        def mix_state(src16, dst, eng, j0=0, j1=J, tr="pe"):
            cp = copy_act if eng == "act" else copy_dve
            cp_rm = copy_dve if eng in ("act", "dve2") else copy_act
            if eng == "dve2":
                cp = copy_dve
            pe_fence(src16[:1, TOFF[j0]:TOFF[j0] + 1])
            js = list(range(j0, j1))
            rms = {}
            for grp in [js[i:i + 3] for i in range(0, len(js), 3)]:
                ng = len(grp)
                rm = hrm.tile([128, 384], F16, tag="hrm", name="hrm")
                pt, k1 = scr_alloc([JT, 128 * ng])
                for gi, j in enumerate(grp):
                    mm(pt[:, gi * 128:gi * 128 + U],
                       src16[:, TOFF[j]:TOFF[j] + JT], w["ident"][:U, :U],
                       start=(gi == 0), stop=(gi == ng - 1))
                scr_done(k1, cp_rm(rm[:JT, :128 * ng], pt[:, :128 * ng]),
                         rm[:1, :1])
                for gi, j in enumerate(grp):
                    rms[j] = (rm, gi * 128)
            for grp in [js[i:i + 2] for i in range(0, len(js), 2)]:
                ng = len(grp)
                pm, k2 = scr_alloc([128, 228 * ng])
                for gi, j in enumerate(grp):
                    rm, off = rms[j]
                    if TSZ[j] == 76:
                        mm(pm[:U, gi * 228:gi * 228 + 152],
                           rm[:JT, off:off + U], w["bd12s"][:, j // 3],
                           start=(gi == 0), stop=(gi == ng - 1))
                    else:
                        mm(pm[:U, gi * 228:(gi + 1) * 228],
                           rm[:JT, off:off + U], w["bd12"][:, j],
                           start=(gi == 0), stop=(gi == ng - 1))
                last = None
                for gi, j in enumerate(grp):
                    wj = 2 * TSZ[j]
                    last = cp(dst[:U, :, TOFF[j]:TOFF[j] + TSZ[j]],
                              pm[:U, gi * 228:gi * 228 + wj]
                              .rearrange("d (m r) -> d m r", m=2))
                scr_done(k2, last, dst[:1, 0, TOFF[grp[-1]]:TOFF[grp[-1]] + 1])
            return dst

        # ------------------------------------------------------------------
        # x input staging for e0: row-major xtile -> mixed feature-major xc
        # ------------------------------------------------------------------
        xtiles = {}
        xcats = {}

        def dma_x(t):
            xt = xpool.tile([JT, J, DIN], F16, tag="xin", name="xin")
            nc.sync.dma_start(xt[:], d["xenc"][t].rearrange("j p q -> p j q"))
            xtiles[t] = xt

        def xstage_a(t):
            xc = xcat.tile([DIN, 3, R], F16, tag="xcat", name="xcat")
            xtile = xtiles.pop(t)
            pe_fence(xtile[:1, 0, :1])
            for j in range(J):
                pm, k = scr_alloc([128, 3 * JT])
                if TSZ[j] == 76:
                    mm(pm[:DIN, :228], xtile[:76, j], w["bdc3s"][:76, j // 3])
                else:
                    mm(pm[:DIN], xtile[:, j], w["bdcat3"][:, j])
                cp = copy_act if j % 2 == 0 else copy_dve
                scr_done(k, cp(xc[:, :, TOFF[j]:TOFF[j] + TSZ[j]],
                               pm[:DIN, :3 * TSZ[j]]
                               .rearrange("d (m r) -> d m r", m=3)),
                         xc[:1, 0, TOFF[j]:TOFF[j] + 1])
            xcats[t] = xc

        # per-half, per-psum-group rolling ACT tokens: a bank may be
        # reclaimed once its ACT reader (sig-r / sig-u / tanh) has run.
        toks = {"r": [None, None], "u": [None, None], "c": [None, None]}
        TOKSLOT = {"r": 0, "u": 2, "c": 4}

        # ------------------------------------------------------------------
        # one DCGRU cell, split into phases so two cells can interleave
        # ------------------------------------------------------------------
        from types import SimpleNamespace

        def cell_mk(cn, xw, xrhs, hsrc, hmix, hdst):
            """xw/xrhs: x-side weight name + rhs fn (None to skip, go-symbol=0).
            hsrc [U,RALL] fp16 prev state; hmix [U,2,R] its P1/P2 mix;
            hdst [U,RALL] fp16 new state (may alias hsrc)."""
            sfx = LAYER_SFX[cn]
            rT, uT, cT, tmp = (
                wk[sfx][k] for k in ("rT", "uT", "cT", "tmp"))
            rh16 = wk[sfx]["rh"]
            st = {"gps": {}, "lastw": {}, "s12rh": None}

            def gmm(hh, c, out, lhsT, rhs, is_last=False):
                key = (hh, c)
                i = mm(out, lhsT, rhs, start=(key not in st["lastw"]),
                       stop=is_last, after=st["lastw"].get(key))
                st["lastw"][key] = i
                return i

            def settok(kind, hh, src_ap):
                # fence target: the ACT instruction's own output slice -- a
                # pe_fence ldweights reading it waits for the activation, which
                # implies its PSUM-bank read is complete.  No token copy.
                toks[kind][hh] = src_ap

            def p_gates_h():
                for hh, (lo, sz, j0, j1) in enumerate(HF):
                    pe_fence(toks["r"][hh])
                    gp0 = pg0.tile([U, 304], F32, tag="gp0", name="gp0")
                    st["gps"][hh] = [gp0, None, None]
                    for m in range(3):
                        rhs = hsrc[:, lo:lo + sz] if m == 0 else hmix[:, m - 1, lo:lo + sz]
                        gmm(hh, 0, gp0[:, :sz], w[f"{cn}_wgh"][:, m, 0:128], rhs,
                            is_last=(m == 2 and xrhs is None))
                for hh, (lo, sz, j0, j1) in enumerate(HF):
                    pe_fence(toks["u"][0], toks["u"][1])
                    gp1 = pg1.tile([U, 304], F32, tag="gp1", name="gp1")
                    st["gps"][hh][1] = gp1
                    for m in range(3):
                        rhs = hsrc[:, lo:lo + sz] if m == 0 else hmix[:, m - 1, lo:lo + sz]
                        gmm(hh, 1, gp1[:, :sz], w[f"{cn}_wgh"][:, m, 128:256], rhs,
                            is_last=(m == 2 and xrhs is None))

            def p_gates_x():
                if xrhs is None:
                    return
                for hh, (lo, sz, j0, j1) in enumerate(HF):
                    for m in range(3):
                        gmm(hh, 0, st["gps"][hh][0][:, :sz], w[xw][:, m, 0:128],
                            xrhs(m)[:, lo:lo + sz], is_last=(m == 2))
                for hh, (lo, sz, j0, j1) in enumerate(HF):
                    for m in range(3):
                        gmm(hh, 1, st["gps"][hh][1][:, :sz], w[xw][:, m, 128:256],
                            xrhs(m)[:, lo:lo + sz], is_last=(m == 2))

            def p_sig_rh():
                s12rh = s12p.tile([U, 2, R], F16, tag=f"s12rh{sfx}", name="s12rh")
                st["s12rh"] = s12rh
                for hh, (lo, sz, j0, j1) in enumerate(HF):
                    nc.scalar.activation(rT[:, lo:lo + sz], st["gps"][hh][0][:, :sz],
                                         AF.Sigmoid, bias=w[f"{cn}_bg"][:, 0:1])
                    settok("r", hh, rT[:1, lo:lo + 1])
                    nc.scalar.activation(uT[:, lo:lo + sz], st["gps"][hh][1][:, :sz],
                                         AF.Sigmoid, bias=w[f"{cn}_bg"][:, 1:2])
                    settok("u", hh, uT[:1, lo:lo + 1])
                    nc.vector.tensor_mul(rh16[:, lo:lo + sz], rT[:, lo:lo + sz],
                                         hsrc[:, lo:lo + sz])
                    mix_state(rh16, s12rh, "dve2", j0, j1, tr="pe")

            def p_cand():
                s12rh = st["s12rh"]
                for hh, (lo, sz, j0, j1) in enumerate(HF):
                    pe_fence(toks["c"][0], toks["c"][1])
                    cpb = pcd.tile([U, 304], F32, tag="cp", name="cp")
                    st["gps"][hh][2] = cpb
                    if xrhs is not None:
                        for m in range(3):
                            gmm(hh, 2, cpb[:, :sz], w[xw][:, m, 256:384],
                                xrhs(m)[:, lo:lo + sz])
                    for m in range(3):
                        rhs = rh16[:, lo:lo + sz] if m == 0 else s12rh[:, m - 1, lo:lo + sz]
                        gmm(hh, 2, cpb[:, :sz], w[f"{cn}_wch"][:, m], rhs,
                            is_last=(m == 2))

            def p_tail_h(hh):
                for hh, (lo, sz, j0, j1) in [(hh, HF[hh])]:
                    sl = slice(lo, lo + sz)
                    nc.scalar.activation(cT[:, sl], st["gps"][hh][2][:, :sz],
                                         AF.Tanh, bias=w[f"{cn}_bc"][:, 0:1])
                    settok("c", hh, cT[:1, lo:lo + 1])
                    eng = nc.gpsimd if sfx == "b" else nc.vector
                    eng.tensor_sub(tmp[:, sl], hsrc[:, sl], cT[:, sl])
                    eng.tensor_mul(tmp[:, sl], uT[:, sl], tmp[:, sl])
                    eng.tensor_add(hdst[:, sl], cT[:, sl], tmp[:, sl])

            def p_tail():
                p_tail_h(0)
                p_tail_h(1)

            return SimpleNamespace(p_gates_h=p_gates_h, p_gates_x=p_gates_x,
                                   p_sig_rh=p_sig_rh, p_cand=p_cand,
                                   p_tail=p_tail, p_tail_h=p_tail_h)

        def cell(cn, xw, xrhs, hsrc, hmix, hdst):
            c = cell_mk(cn, xw, xrhs, hsrc, hmix, hdst)
            c.p_gates_h(); c.p_gates_x(); c.p_sig_rh(); c.p_cand(); c.p_tail()

        # ------------------------------------------------------------------
        # encoder: e1 runs one step behind e0, phase-interleaved
        # ------------------------------------------------------------------
        m0_tiles = {}

        def mk_mix0(t, tr):
            mt = m0pool.tile([U, 2, R], F16, tag="m0", name="m0")
            mix_state(h0seq[:, t], mt, "act", 0, J, tr=tr)
            m0_tiles[t] = mt

        def e1_mk(tt):
            return cell_mk("e1", "e1_wx",
                           lambda m, tt=tt: h0seq[:, tt] if m == 0
                           else m0_tiles[tt][:, m - 1],
                           hsrc=(hz16 if tt == 0 else h1T), hmix=m1, hdst=h1T)

        def e0_mk(t):
            xc = xcats.pop(t)
            return cell_mk("e0", "e0_wx", lambda m, xc=xc: xc[:, m],
                           hsrc=(hz16 if t == 0 else h0seq[:, t - 1]),
                           hmix=(zmix if t == 0 else m0_tiles[t - 1]),
                           hdst=h0seq[:, t])

        dma_x(0)
        if t_enc > 1:
            dma_x(1)
        xstage_a(0)
        A = e0_mk(0)
        A.p_gates_h(); A.p_gates_x()
        for t in range(t_enc):
            if t + 2 < t_enc:
                dma_x(t + 2)
            if t + 1 < t_enc:
                xstage_a(t + 1)
            A.p_sig_rh()
            B = e1_mk(t - 1) if t >= 1 else None
            if B:
                B.p_gates_h(); B.p_gates_x()
            A.p_cand()
            if B:
                B.p_sig_rh()
            A.p_tail_h(0)
            if B:
                B.p_cand()
            mt = m0pool.tile([U, 2, R], F16, tag="m0", name="m0")
            m0_tiles[t] = mt
            mix_state(h0seq[:, t], mt, "act", 0, 3, tr="pe")
            A.p_tail_h(1)
            mix_state(h0seq[:, t], mt, "act", 3, 6, tr="pe")
            # software pipeline: next e0's h-side gate matmuls fill the
            # e1-tail / mix1 stall window (their inputs are ready here)
            if t + 1 < t_enc:
                An = e0_mk(t + 1)
                An.p_gates_h(); An.p_gates_x()
            if B:
                B.p_tail()
                mix_state(h1T, m1, "act", 0, J, tr="pe")
            if t + 1 < t_enc:
                A = An
        B = e1_mk(t_enc - 1)
        B.p_gates_h(); B.p_gates_x(); B.p_sig_rh(); B.p_cand(); B.p_tail()
        mix_state(h1T, m1, "act", 0, J, tr="pe")

        # ------------------------------------------------------------------
        # decoder: d0 x-side folded through WW on (h1, m1); proj output-only.
        # d1's h-side gate matmuls hoisted into d0's stall window.
        # ------------------------------------------------------------------
        def d0_mk(s):
            if s == 0:
                return cell_mk("d0", None, None,
                               hsrc=h0seq[:, t_enc - 1],
                               hmix=m0_tiles[t_enc - 1], hdst=hd0)
            return cell_mk("d0", "d0_ww",
                           lambda m: h1T if m == 0 else m1[:, m - 1],
                           hsrc=hd0, hmix=m0d, hdst=hd0)

        D0 = d0_mk(0)
        D0.p_gates_h()
        for s in range(t_dec):
            D1 = cell_mk("d1", "d1_wx",
                         lambda m: hd0 if m == 0 else m0d[:, m - 1],
                         hsrc=h1T, hmix=m1, hdst=h1T)
            D0.p_gates_x(); D0.p_sig_rh()
            D1.p_gates_h()
            D0.p_cand()
            D0.p_tail_h(0)
            mix_state(hd0, m0d, "dve", 0, 3, tr="pe")
            D0.p_tail_h(1)
            mix_state(hd0, m0d, "dve", 3, 6, tr="pe")
            D1.p_gates_x(); D1.p_sig_rh()
            # software pipeline: next d0's h-side gate matmuls (hd0_s and
            # m0d_s are ready) fill d1's cand/tail stall window
            if s + 1 < t_dec:
                D0n = d0_mk(s + 1)
                D0n.p_gates_h()
            D1.p_cand(); D1.p_tail()
            if s + 1 < t_dec:
                D0 = D0n
            # projection (output only, off critical path)
            pe_fence(h1T[:1, :1])
            for lo, sz, j0, j1 in HF:
                pp, k = scr_alloc([O, 304])
                mm(pp[:, :sz], w["wp"][:], h1T[:, lo:lo + sz])
                nc.scalar.activation(projT32[:, lo:lo + sz], pp[:, :sz],
                                     AF.Copy)
                scr_done(k, copy_act(tokT[:1, 6:7], projT32[:1, lo:lo + 1]),
                         tokT[:1, 6:7])
            nc.vector.tensor_scalar_add(projT32[:], projT32[:], w["bp"][:, 0:1])
            nc.sync.dma_start(d["out"][s], projT32[:])
            mix_state(h1T, m1, "act", 0, J, tr="pe")



# --------------------------------------------------------------------------
# entry point
# --------------------------------------------------------------------------

def kernel(**inputs):
    arrs = _prep_host(inputs)
    nc = build_program(T, T)
    in_maps = []
    for core in range(NCORES):
        m = dict(arrs)
        m["xenc"] = _prep_xenc(inputs["encoder_inputs"], core)
        in_maps.append(m)
    res = run_bass_kernel_spmd(nc, in_maps, list(range(NCORES))).results
    outs = []
    for core in range(NCORES):
        o = np.asarray(res[core]["out"], np.float32)[:, :, :RV]   # [T, O, RV]
        o = o.reshape(T, O, BC, N).transpose(2, 0, 3, 1)          # [BC, T, N, O]
        outs.append(o)
    return np.ascontiguousarray(np.concatenate(outs, axis=0))



# revision 6
# speedup vs baseline: 1.0470x; 1.0470x over previous
"""DCRNN (nn_DCRNNModel_nextTimePred) Bass/Tile kernel for 8 TRN2 NeuronCores, v2.

Sharding: data-parallel over batch (B=256 -> 32 per core), weights replicated.
Layout: activations feature-major [feat<=128, R] with R = 32*19 = 608 rows
zero-padded to 684 = 6 tiles x 114 (6 batch x 19 nodes per tile).

v2 structural changes vs v1:
- fp16 state + fp16 elementwise GRU tail (DVE 2x mode), no separate casts.
- Each layer's node-mix (BD-matmul of its fresh h) is computed ONCE per step
  and shared by (a) that layer's own gates next step and (b) the next layer's
  x-side this step.  m=0 term is the state tensor itself.
- Decoder projection feedback folded into the weights: (P_k(h1 Wp))Wx_k =
  (P_k h1)(Wp Wx_k), so d0's x-side consumes the already-computed mix of h1
  with combined weights WW_k = Wp @ Wx_k.  proj is computed only for output,
  off the critical path.  (bp = 0 for this problem so no bias correction.)
- Gate/cand matmuls at N=342 (full PSUM half) instead of N=114.
- State mixes use DMA transposes (cross-step latency tolerant); the rh mix and
  the decoder's h0-mix (same-step consumers) use PE transposes.  Transpose and
  mix outputs are paired two-j-tiles-per-PSUM-bank to halve copy count.
"""
import numpy as np

import concourse.bass as bass
import concourse.mybir as mybir
from concourse import bacc, tile
from concourse.bass_utils import run_bass_kernel_spmd

F16 = mybir.dt.float16
F32 = mybir.dt.float32
AF = mybir.ActivationFunctionType

NCORES = 8
B, T, N, DIN, U, O = 256, 64, 19, 100, 128, 100
BC = B // NCORES          # 32 batch per core
JT = 114                  # transpose window width (max tile rows)
J = 6                     # row tiles
R = 608                   # compact rows (no padding)
RV = BC * N               # valid rows = 608
HF = ((0, 304, 0, 3), (304, 304, 3, 6))  # half-chunks: (lo, sz, j0, j1)
RALL = 648   # transpose windows read [off, off+114) <= 646; zeros beyond 608
TOFF = (0, 114, 228, 304, 418, 532)      # compact tile offsets
TSZ = (114, 114, 76, 114, 114, 76)       # tile row counts
TB0 = (0, 6, 12, 16, 22, 28)             # first batch per tile
TNB = (6, 6, 4, 6, 6, 4)                 # batches per tile

CELLS = ("e0", "e1", "d0", "d1")
CELL_DIN = {"e0": DIN, "e1": U, "d0": DIN, "d1": U}


def _pack_layouts():
    p16, off = [], 0
    def add16(name, p, shape):
        nonlocal off
        n = int(np.prod(shape))
        p16.append((name, p, tuple(shape), off))
        off += n
    add16("bdcat3", JT, (J, 3 * JT))
    add16("bd12", JT, (J, 2 * JT))
    add16("bd12s", JT, (2, 152))
    add16("bdc3s", JT, (2, 228))
    add16("ident", 128, (128,))
    add16("wp", U, (O,))
    add16("d0_ww", U, (3, 384))
    for c in CELLS:
        add16(f"{c}_wx", CELL_DIN[c], (3, 384))
        add16(f"{c}_wgh", U, (3, 256))
        add16(f"{c}_wch", U, (3, 128))
    f16_total = off
    p32, off = [], 0
    def add32(name, p, shape):
        nonlocal off
        n = int(np.prod(shape))
        p32.append((name, p, tuple(shape), off))
        off += n
    for c in CELLS:
        add32(f"{c}_bg", U, (2,))
        add32(f"{c}_bgn", U, (1,))
        add32(f"{c}_bc", U, (1,))
    add32("bp", O, (1,))
    return p16, f16_total, p32, off


PACK16, F16TOT, PACK32, F32TOT = _pack_layouts()


# --------------------------------------------------------------------------
# host-side weight preparation
# --------------------------------------------------------------------------

def _prep_host(inputs):
    f32 = np.float32
    S = np.asarray(inputs["support"], f32)
    P = [np.eye(N, dtype=f32), S, 2.0 * (S @ S) - np.eye(N, dtype=f32)]

    def bd_t(m, nb):
        Z = np.zeros((JT, JT), f32)
        for b in range(nb):
            Z[b * N:(b + 1) * N, b * N:(b + 1) * N] = P[m].T
        return Z

    vals = {}
    bdcat3 = np.zeros((JT, J, 3 * JT), f32)
    bd12 = np.zeros((JT, J, 2 * JT), f32)
    for j in range(J):
        nb = TNB[j]
        eye = np.zeros((JT, JT), f32)
        eye[:nb * N, :nb * N] = np.eye(nb * N, dtype=f32)
        bdcat3[:, j] = np.concatenate([eye, bd_t(1, nb), bd_t(2, nb)], axis=1)
        bd12[:, j] = np.concatenate([bd_t(1, nb), bd_t(2, nb)], axis=1)
    vals["bdcat3"] = bdcat3
    vals["bd12"] = bd12
    bd12s = np.zeros((JT, 2, 152), f32)
    bdc3s = np.zeros((JT, 2, 228), f32)
    for i, j in enumerate((2, 5)):
        bd12s[:, i] = np.concatenate(
            [bd_t(1, 4)[:, :76], bd_t(2, 4)[:, :76]], axis=1)
        ey = np.zeros((JT, 76), f32)
        ey[:76] = np.eye(76, dtype=f32)
        bdc3s[:, i] = np.concatenate(
            [ey, bd_t(1, 4)[:, :76], bd_t(2, 4)[:, :76]], axis=1)
    vals["bd12s"] = bd12s
    vals["bdc3s"] = bdc3s
    vals["ident"] = np.eye(128, dtype=f32)

    for c in CELLS:
        din = CELL_DIN[c]
        Wg = np.asarray(inputs[f"{c}_Wg"], f32)   # [(din+U)*3, 2U]
        Wc = np.asarray(inputs[f"{c}_Wc"], f32)   # [(din+U)*3, U]
        wx, wgh, wch = [], [], []
        for m in range(3):
            Wg_m, Wc_m = Wg[m::3], Wc[m::3]       # [(din+U), .]
            wx.append(np.concatenate([Wg_m[:din], Wc_m[:din]], axis=1))  # [din,384]
            wgh.append(Wg_m[din:])                # [U, 256]
            wch.append(Wc_m[din:])                # [U, 128]
        vals[f"{c}_wx"] = np.stack(wx, axis=1)    # [din, 3, 384]
        vals[f"{c}_wgh"] = np.stack(wgh, axis=1)  # [U, 3, 256]
        vals[f"{c}_wch"] = np.stack(wch, axis=1)  # [U, 3, 128]
        bg = np.asarray(inputs[f"{c}_bg"], f32)
        vals[f"{c}_bg"] = np.stack([bg[:U], bg[U:]], axis=1)  # [U, 2]
        vals[f"{c}_bgn"] = -bg[U:].reshape(U, 1)              # for 1-u = sig(-z)
        vals[f"{c}_bc"] = np.asarray(inputs[f"{c}_bc"], f32).reshape(U, 1)
    Wp = np.asarray(inputs["Wp"], f32)
    vals["wp"] = Wp                               # [U, O]
    vals["bp"] = np.asarray(inputs["bp"], f32).reshape(O, 1)
    # decoder projection-feedback fold: WW_m = Wp @ wx_m(d0)  [U, 384]
    vals["d0_ww"] = np.stack([Wp @ vals["d0_wx"][:, m] for m in range(3)], axis=1)
    assert np.abs(np.asarray(inputs["bp"], f32)).max() == 0.0, \
        "nonzero bp needs the rank-3 bias correction (not emitted)"

    pack16 = np.zeros((128, F16TOT), np.float16)
    for name, p, shape, off in PACK16:
        n = int(np.prod(shape))
        pack16[:p, off:off + n] = vals[name].reshape(p, n).astype(np.float16)
    pack32 = np.zeros((128, F32TOT), f32)
    for name, p, shape, off in PACK32:
        n = int(np.prod(shape))
        pack32[:p, off:off + n] = vals[name].reshape(p, n)
    return {"wpack16": pack16, "wpack32": pack32}


def _prep_xenc(enc, core, t_enc=T):
    """per-core encoder input -> [T, J, JT, DIN] fp16, zero row padding."""
    e = np.asarray(enc[core * BC:(core + 1) * BC, :t_enc], np.float32)
    e = e.transpose(1, 0, 2, 3)                                 # [T, BC, N, DIN]
    out = np.zeros((t_enc, J, JT, DIN), np.float16)
    for j in range(J):
        out[:, j, :TNB[j] * N] = \
            e[:, TB0[j]:TB0[j] + TNB[j]].reshape(t_enc, TNB[j] * N, DIN)
    return out


# --------------------------------------------------------------------------
# program builder
# --------------------------------------------------------------------------

def build_program(t_enc=T, t_dec=T):
    nc = bacc.Bacc()
    d = {}
    d["xenc"] = nc.dram_tensor("xenc", [t_enc, J, JT, DIN], F16, kind="ExternalInput")
    d["wpack16"] = nc.dram_tensor("wpack16", [128, F16TOT], F16, kind="ExternalInput")
    d["wpack32"] = nc.dram_tensor("wpack32", [128, F32TOT], F32, kind="ExternalInput")
    d["out"] = nc.dram_tensor("out", [t_dec, O, R], F32, kind="ExternalOutput")
    with tile.TileContext(nc) as tc:
        _emit(nc, tc, d, t_enc, t_dec)
    nc.finalize()
    return nc


def _emit(nc, tc, d, t_enc, t_dec):
    import contextlib
    stack = contextlib.ExitStack()
    with stack:
        perm = stack.enter_context(tc.tile_pool(name="perm", bufs=1))
        xpool = stack.enter_context(tc.tile_pool(name="xin", bufs=3))
        xcat = stack.enter_context(tc.tile_pool(name="xcat", bufs=3))
        m0pool = stack.enter_context(tc.tile_pool(name="m0p", bufs=4))
        s12p = stack.enter_context(tc.tile_pool(name="s12", bufs=3))
        hrm = stack.enter_context(tc.tile_pool(name="hrm", bufs=6))
        pg0 = stack.enter_context(tc.tile_pool(name="pg0", bufs=2, space="PSUM"))
        pg1 = stack.enter_context(tc.tile_pool(name="pg1", bufs=1, space="PSUM"))
        pcd = stack.enter_context(tc.tile_pool(name="pcd", bufs=1, space="PSUM"))
        SCR_BUFS = 4
        scr = stack.enter_context(tc.tile_pool(name="scr", bufs=SCR_BUFS, space="PSUM"))

        # ---- persistent weights ----
        wp16 = perm.tile([128, F16TOT], F16, name="wp16", tag="wp16")
        wp32 = perm.tile([128, F32TOT], F32, name="wp32", tag="wp32")
        nc.sync.dma_start(wp16[:], d["wpack16"][:])
        nc.sync.dma_start(wp32[:], d["wpack32"][:])
        w = {}
        for name, p, shape, off in PACK16:
            n = int(np.prod(shape))
            ap = wp16[:p, off:off + n]
            if len(shape) > 1:
                ap = ap.rearrange("p (a b) -> p a b", a=shape[0])
            w[name] = ap
        for name, p, shape, off in PACK32:
            n = int(np.prod(shape))
            w[name] = wp32[:p, off:off + n]

        # ---- persistent state (all fp16) ----
        h0seq = perm.tile([U, t_enc, RALL], F16, tag="h0seq")   # h0 after step t
        h1T = perm.tile([U, RALL], F16, tag="h1T")              # h1 state (enc+dec)
        hd0 = perm.tile([U, RALL], F16, tag="hd0")              # decoder h0 state
        hz16 = perm.tile([U, RALL], F16, tag="hz16")            # zeros
        zmix = perm.tile([U, 2, R], F16, tag="zmix")            # zero mix
        m1 = perm.tile([U, 2, R], F16, tag="m1")                # mix of h1 (P1,P2)
        m0d = perm.tile([U, 2, R], F16, tag="m0d")              # decoder mix of h0
        for tl in (h0seq, h1T, hd0, hz16, zmix, m1, m0d):
            nc.gpsimd.memset(tl[:], 0.0)

        # per-layer elementwise work tiles (a: layer0 cells, b: layer1 cells)
        wk = {}
        for sfx in ("a", "b"):
            wk[sfx] = {
                k: perm.tile([U, RALL if k == "rh" else R], F16,
                             tag=f"{k}{sfx}", name=f"{k}{sfx}")
                for k in ("rT", "uT", "cT", "tmp", "rh")
            }
            nc.gpsimd.memset(wk[sfx]["rh"][:], 0.0)
        LAYER_SFX = {"e0": "a", "d0": "a", "e1": "b", "d1": "b"}

        projT32 = perm.tile([O, R], F32, tag="projT32")
        tokT = perm.tile([1, 8], F16, tag="tokT")

        # ------------------------------------------------------------------
        # one-wait-per-matmul machinery (see v1 notes): pe_fence plants tiny
        # LDWEIGHTS wait-carriers; mm ties matmuls behind the latest fence.
        # ------------------------------------------------------------------
        from concourse.tile import add_dep_helper

        cur_fence = [None]

        def _raw(i):
            return i.ins if hasattr(i, "ins") and not isinstance(i.ins, list) else i

        def pe_fence(*tok_aps):
            for tok in tok_aps:
                if tok is None:
                    continue
                i = nc.tensor.ldweights(weights=tok)
                if cur_fence[0] is not None:
                    add_dep_helper(_raw(i), _raw(cur_fence[0]), sync=False)
                cur_fence[0] = i

        def mm(*args, after=None, **kw):
            i = nc.tensor.matmul(*args, **kw)
            if cur_fence[0] is not None:
                add_dep_helper(_raw(i), _raw(cur_fence[0]), sync=False)
            if after is not None:
                add_dep_helper(_raw(i), _raw(after), sync=False)
            return i

        def copy_act(dst, src):
            return nc.scalar.activation(dst, src, AF.Copy)

        def copy_dve(dst, src):
            return nc.vector.tensor_copy(dst, src)

        scr_state = {"k": 0, "toks": {}}

        def scr_alloc(shape):
            k = scr_state["k"]
            scr_state["k"] += 1
            tok = scr_state["toks"].get(k - SCR_BUFS)
            if tok is not None:
                pe_fence(tok)
            return scr.tile(shape, F32, tag="scr", name="scr"), k

        def scr_done(k, reader_instr, tok):
            scr_state["toks"][k] = tok
            return reader_instr

        # ------------------------------------------------------------------
        # node-mix of a feature-major fp16 tensor: src [U, RALL] -> dst [U,2,R]
        # (P1, P2 diffusion terms).  tr='pe': PE transpose (low latency);
        # tr='dma': DMA-xbar transpose (no PE/copy cost, ~us latency).
        # j-tiles are paired two-per-PSUM-bank to halve copy count.
        # ------------------------------------------------------------------
        def mix_state(src16, dst, eng, j0=0, j1=J, tr="pe"):
            cp = copy_act if eng == "act" else copy_dve
            cp_rm = copy_dve if eng in ("act", "dve2") else copy_act
            if eng == "dve2":
                cp = copy_dve
            pe_fence(src16[:1, TOFF[j0]:TOFF[j0] + 1])
            js = list(range(j0, j1))
            rms = {}
            for grp in [js[i:i + 3] for i in range(0, len(js), 3)]:
                ng = len(grp)
                rm = hrm.tile([128, 384], F16, tag="hrm", name="hrm")
                pt, k1 = scr_alloc([JT, 128 * ng])
                for gi, j in enumerate(grp):
                    mm(pt[:, gi * 128:gi * 128 + U],
                       src16[:, TOFF[j]:TOFF[j] + JT], w["ident"][:U, :U],
                       start=(gi == 0), stop=(gi == ng - 1))
                scr_done(k1, cp_rm(rm[:JT, :128 * ng], pt[:, :128 * ng]),
                         rm[:1, :1])
                for gi, j in enumerate(grp):
                    rms[j] = (rm, gi * 128)
            for grp in [js[i:i + 2] for i in range(0, len(js), 2)]:
                ng = len(grp)
                pm, k2 = scr_alloc([128, 228 * ng])
                for gi, j in enumerate(grp):
                    rm, off = rms[j]
                    if TSZ[j] == 76:
                        mm(pm[:U, gi * 228:gi * 228 + 152],
                           rm[:JT, off:off + U], w["bd12s"][:, j // 3],
                           start=(gi == 0), stop=(gi == ng - 1))
                    else:
                        mm(pm[:U, gi * 228:(gi + 1) * 228],
                           rm[:JT, off:off + U], w["bd12"][:, j],
                           start=(gi == 0), stop=(gi == ng - 1))
                last = None
                for gi, j in enumerate(grp):
                    wj = 2 * TSZ[j]
                    last = cp(dst[:U, :, TOFF[j]:TOFF[j] + TSZ[j]],
                              pm[:U, gi * 228:gi * 228 + wj]
                              .rearrange("d (m r) -> d m r", m=2))
                scr_done(k2, last, dst[:1, 0, TOFF[grp[-1]]:TOFF[grp[-1]] + 1])
            return dst

        # ------------------------------------------------------------------
        # x input staging for e0: row-major xtile -> mixed feature-major xc
        # ------------------------------------------------------------------
        xtiles = {}
        xcats = {}

        def dma_x(t):
            xt = xpool.tile([JT, J, DIN], F16, tag="xin", name="xin")
            nc.sync.dma_start(xt[:], d["xenc"][t].rearrange("j p q -> p j q"))
            xtiles[t] = xt

        def xstage_a(t):
            xc = xcat.tile([DIN, 3, R], F16, tag="xcat", name="xcat")
            xtile = xtiles.pop(t)
            pe_fence(xtile[:1, 0, :1])
            for j in range(J):
                pm, k = scr_alloc([128, 3 * JT])
                if TSZ[j] == 76:
                    mm(pm[:DIN, :228], xtile[:76, j], w["bdc3s"][:76, j // 3])
                else:
                    mm(pm[:DIN], xtile[:, j], w["bdcat3"][:, j])
                cp = copy_act if j % 2 == 0 else copy_dve
                scr_done(k, cp(xc[:, :, TOFF[j]:TOFF[j] + TSZ[j]],
                               pm[:DIN, :3 * TSZ[j]]
                               .rearrange("d (m r) -> d m r", m=3)),
                         xc[:1, 0, TOFF[j]:TOFF[j] + 1])
            xcats[t] = xc

        # per-half, per-psum-group rolling ACT tokens: a bank may be
        # reclaimed once its ACT reader (sig-r / sig-u / tanh) has run.
        toks = {"r": [None, None], "u": [None, None], "c": [None, None]}
        TOKSLOT = {"r": 0, "u": 2, "c": 4}

        # ------------------------------------------------------------------
        # one DCGRU cell, split into phases so two cells can interleave
        # ------------------------------------------------------------------
        from types import SimpleNamespace

        def cell_mk(cn, xw, xrhs, hsrc, hmix, hdst):
            """xw/xrhs: x-side weight name + rhs fn (None to skip, go-symbol=0).
            hsrc [U,RALL] fp16 prev state; hmix [U,2,R] its P1/P2 mix;
            hdst [U,RALL] fp16 new state (may alias hsrc)."""
            sfx = LAYER_SFX[cn]
            rT, uT, cT, tmp = (
                wk[sfx][k] for k in ("rT", "uT", "cT", "tmp"))
            rh16 = wk[sfx]["rh"]
            st = {"gps": {}, "lastw": {}, "s12rh": None}

            def gmm(hh, c, out, lhsT, rhs, is_last=False):
                key = (hh, c)
                i = mm(out, lhsT, rhs, start=(key not in st["lastw"]),
                       stop=is_last, after=st["lastw"].get(key))
                st["lastw"][key] = i
                return i

            def settok(kind, hh, src_ap):
                # fence target: the ACT instruction's own output slice -- a
                # pe_fence ldweights reading it waits for the activation, which
                # implies its PSUM-bank read is complete.  No token copy.
                toks[kind][hh] = src_ap

            def p_gates_h():
                for hh, (lo, sz, j0, j1) in enumerate(HF):
                    pe_fence(toks["r"][hh])
                    gp0 = pg0.tile([U, 304], F32, tag="gp0", name="gp0")
                    st["gps"][hh] = [gp0, None, None]
                    for m in range(3):
                        rhs = hsrc[:, lo:lo + sz] if m == 0 else hmix[:, m - 1, lo:lo + sz]
                        gmm(hh, 0, gp0[:, :sz], w[f"{cn}_wgh"][:, m, 0:128], rhs,
                            is_last=(m == 2 and xrhs is None))
                for hh, (lo, sz, j0, j1) in enumerate(HF):
                    pe_fence(toks["u"][0], toks["u"][1])
                    gp1 = pg1.tile([U, 304], F32, tag="gp1", name="gp1")
                    st["gps"][hh][1] = gp1
                    for m in range(3):
                        rhs = hsrc[:, lo:lo + sz] if m == 0 else hmix[:, m - 1, lo:lo + sz]
                        gmm(hh, 1, gp1[:, :sz], w[f"{cn}_wgh"][:, m, 128:256], rhs,
                            is_last=(m == 2 and xrhs is None))

            def p_gates_x():
                if xrhs is None:
                    return
                for hh, (lo, sz, j0, j1) in enumerate(HF):
                    for m in range(3):
                        gmm(hh, 0, st["gps"][hh][0][:, :sz], w[xw][:, m, 0:128],
                            xrhs(m)[:, lo:lo + sz], is_last=(m == 2))
                for hh, (lo, sz, j0, j1) in enumerate(HF):
                    for m in range(3):
                        gmm(hh, 1, st["gps"][hh][1][:, :sz], w[xw][:, m, 128:256],
                            xrhs(m)[:, lo:lo + sz], is_last=(m == 2))

            def p_sig_rh():
                s12rh = s12p.tile([U, 2, R], F16, tag=f"s12rh{sfx}", name="s12rh")
                st["s12rh"] = s12rh
                for hh, (lo, sz, j0, j1) in enumerate(HF):
                    nc.scalar.activation(rT[:, lo:lo + sz], st["gps"][hh][0][:, :sz],
                                         AF.Sigmoid, bias=w[f"{cn}_bg"][:, 0:1])
                    settok("r", hh, rT[:1, lo:lo + 1])
                    nc.scalar.activation(uT[:, lo:lo + sz], st["gps"][hh][1][:, :sz],
                                         AF.Sigmoid, bias=w[f"{cn}_bg"][:, 1:2])
                    settok("u", hh, uT[:1, lo:lo + 1])
                    nc.vector.tensor_mul(rh16[:, lo:lo + sz], rT[:, lo:lo + sz],
                                         hsrc[:, lo:lo + sz])
                    mix_state(rh16, s12rh, "dve2", j0, j1, tr="pe")

            def p_cand():
                s12rh = st["s12rh"]
                for hh, (lo, sz, j0, j1) in enumerate(HF):
                    pe_fence(toks["c"][0], toks["c"][1])
                    cpb = pcd.tile([U, 304], F32, tag="cp", name="cp")
                    st["gps"][hh][2] = cpb
                    if xrhs is not None:
                        for m in range(3):
                            gmm(hh, 2, cpb[:, :sz], w[xw][:, m, 256:384],
                                xrhs(m)[:, lo:lo + sz])
                    for m in range(3):
                        rhs = rh16[:, lo:lo + sz] if m == 0 else s12rh[:, m - 1, lo:lo + sz]
                        gmm(hh, 2, cpb[:, :sz], w[f"{cn}_wch"][:, m], rhs,
                            is_last=(m == 2))

            def p_tail_h(hh):
                for hh, (lo, sz, j0, j1) in [(hh, HF[hh])]:
                    sl = slice(lo, lo + sz)
                    nc.scalar.activation(cT[:, sl], st["gps"][hh][2][:, :sz],
                                         AF.Tanh, bias=w[f"{cn}_bc"][:, 0:1])
                    settok("c", hh, cT[:1, lo:lo + 1])
                    nc.vector.tensor_sub(tmp[:, sl], hsrc[:, sl], cT[:, sl])
                    nc.vector.tensor_mul(tmp[:, sl], uT[:, sl], tmp[:, sl])
                    nc.vector.tensor_add(hdst[:, sl], cT[:, sl], tmp[:, sl])

            def p_tail():
                p_tail_h(0)
                p_tail_h(1)

            return SimpleNamespace(p_gates_h=p_gates_h, p_gates_x=p_gates_x,
                                   p_sig_rh=p_sig_rh, p_cand=p_cand,
                                   p_tail=p_tail, p_tail_h=p_tail_h)

        def cell(cn, xw, xrhs, hsrc, hmix, hdst):
            c = cell_mk(cn, xw, xrhs, hsrc, hmix, hdst)
            c.p_gates_h(); c.p_gates_x(); c.p_sig_rh(); c.p_cand(); c.p_tail()

        # ------------------------------------------------------------------
        # encoder: e1 runs one step behind e0, phase-interleaved
        # ------------------------------------------------------------------
        m0_tiles = {}

        def mk_mix0(t, tr):
            mt = m0pool.tile([U, 2, R], F16, tag="m0", name="m0")
            mix_state(h0seq[:, t], mt, "act", 0, J, tr=tr)
            m0_tiles[t] = mt

        def e1_mk(tt):
            return cell_mk("e1", "e1_wx",
                           lambda m, tt=tt: h0seq[:, tt] if m == 0
                           else m0_tiles[tt][:, m - 1],
                           hsrc=(hz16 if tt == 0 else h1T), hmix=m1, hdst=h1T)

        def e0_mk(t):
            xc = xcats.pop(t)
            return cell_mk("e0", "e0_wx", lambda m, xc=xc: xc[:, m],
                           hsrc=(hz16 if t == 0 else h0seq[:, t - 1]),
                           hmix=(zmix if t == 0 else m0_tiles[t - 1]),
                           hdst=h0seq[:, t])

        dma_x(0)
        if t_enc > 1:
            dma_x(1)
        xstage_a(0)
        A = e0_mk(0)
        A.p_gates_h(); A.p_gates_x()
        for t in range(t_enc):
            if t + 2 < t_enc:
                dma_x(t + 2)
            if t + 1 < t_enc:
                xstage_a(t + 1)
            A.p_sig_rh()
            B = e1_mk(t - 1) if t >= 1 else None
            if B:
                B.p_gates_h(); B.p_gates_x()
            A.p_cand()
            if B:
                B.p_sig_rh()
            A.p_tail_h(0)
            if B:
                B.p_cand()
            mt = m0pool.tile([U, 2, R], F16, tag="m0", name="m0")
            m0_tiles[t] = mt
            mix_state(h0seq[:, t], mt, "act", 0, 3, tr="pe")
            A.p_tail_h(1)
            mix_state(h0seq[:, t], mt, "act", 3, 6, tr="pe")
            # software pipeline: next e0's h-side gate matmuls fill the
            # e1-tail / mix1 stall window (their inputs are ready here)
            if t + 1 < t_enc:
                An = e0_mk(t + 1)
                An.p_gates_h(); An.p_gates_x()
            if B:
                B.p_tail()
                mix_state(h1T, m1, "act", 0, J, tr="pe")
            if t + 1 < t_enc:
                A = An
        B = e1_mk(t_enc - 1)
        B.p_gates_h(); B.p_gates_x(); B.p_sig_rh(); B.p_cand(); B.p_tail()
        mix_state(h1T, m1, "act", 0, J, tr="pe")

        # ------------------------------------------------------------------
        # decoder: d0 x-side folded through WW on (h1, m1); proj output-only.
        # d1's h-side gate matmuls hoisted into d0's stall window.
        # ------------------------------------------------------------------
        def d0_mk(s):
            if s == 0:
                return cell_mk("d0", None, None,
                               hsrc=h0seq[:, t_enc - 1],
                               hmix=m0_tiles[t_enc - 1], hdst=hd0)
            return cell_mk("d0", "d0_ww",
                           lambda m: h1T if m == 0 else m1[:, m - 1],
                           hsrc=hd0, hmix=m0d, hdst=hd0)

        D0 = d0_mk(0)
        D0.p_gates_h()
        for s in range(t_dec):
            D1 = cell_mk("d1", "d1_wx",
                         lambda m: hd0 if m == 0 else m0d[:, m - 1],
                         hsrc=h1T, hmix=m1, hdst=h1T)
            D0.p_gates_x(); D0.p_sig_rh()
            D1.p_gates_h()
            D0.p_cand()
            D0.p_tail_h(0)
            mix_state(hd0, m0d, "dve", 0, 3, tr="pe")
            D0.p_tail_h(1)
            mix_state(hd0, m0d, "dve", 3, 6, tr="pe")
            D1.p_gates_x(); D1.p_sig_rh()
            # software pipeline: next d0's h-side gate matmuls (hd0_s and
            # m0d_s are ready) fill d1's cand/tail stall window
            if s + 1 < t_dec:
                D0n = d0_mk(s + 1)
                D0n.p_gates_h()
            D1.p_cand(); D1.p_tail()
            if s + 1 < t_dec:
                D0 = D0n
            # projection (output only, off critical path)
            pe_fence(h1T[:1, :1])
            for lo, sz, j0, j1 in HF:
                pp, k = scr_alloc([O, 304])
                mm(pp[:, :sz], w["wp"][:], h1T[:, lo:lo + sz])
                nc.scalar.activation(projT32[:, lo:lo + sz], pp[:, :sz],
                                     AF.Copy)
                scr_done(k, copy_act(tokT[:1, 6:7], projT32[:1, lo:lo + 1]),
                         tokT[:1, 6:7])
            nc.vector.tensor_scalar_add(projT32[:], projT32[:], w["bp"][:, 0:1])
            nc.sync.dma_start(d["out"][s], projT32[:])
            mix_state(h1T, m1, "act", 0, J, tr="pe")



# --------------------------------------------------------------------------
# entry point
# --------------------------------------------------------------------------

def kernel(**inputs):
    arrs = _prep_host(inputs)
    nc = build_program(T, T)
    in_maps = []
    for core in range(NCORES):
        m = dict(arrs)
        m["xenc"] = _prep_xenc(inputs["encoder_inputs"], core)
        in_maps.append(m)
    res = run_bass_kernel_spmd(nc, in_maps, list(range(NCORES))).results
    outs = []
    for core in range(NCORES):
        o = np.asarray(res[core]["out"], np.float32)[:, :, :RV]   # [T, O, RV]
        o = o.reshape(T, O, BC, N).transpose(2, 0, 3, 1)          # [BC, T, N, O]
        outs.append(o)
    return np.ascontiguousarray(np.concatenate(outs, axis=0))



# revision 7
# speedup vs baseline: 1.0620x; 1.0144x over previous
"""DCRNN (nn_DCRNNModel_nextTimePred) Bass/Tile kernel for 8 TRN2 NeuronCores, v2.

Sharding: data-parallel over batch (B=256 -> 32 per core), weights replicated.
Layout: activations feature-major [feat<=128, R] with R = 32*19 = 608 rows
compact 608 rows in 6 tiles [114,114,76|114,114,76] aligned to 304-halves.

v2 structural changes vs v1:
- fp16 state + fp16 elementwise GRU tail (DVE 2x mode), no separate casts.
- Each layer's node-mix (BD-matmul of its fresh h) is computed ONCE per step
  and shared by (a) that layer's own gates next step and (b) the next layer's
  x-side this step.  m=0 term is the state tensor itself.
- Decoder projection feedback folded into the weights: (P_k(h1 Wp))Wx_k =
  (P_k h1)(Wp Wx_k), so d0's x-side consumes the already-computed mix of h1
  with combined weights WW_k = Wp @ Wx_k.  proj is computed only for output,
  off the critical path.  (bp = 0 for this problem so no bias correction.)
- Gate/cand matmuls at N=304 halves (compact rows, no padding).
- State mixes use DMA transposes (cross-step latency tolerant); the rh mix and
  the decoder's h0-mix (same-step consumers) use PE transposes.  Transpose and
  mix outputs are paired two-j-tiles-per-PSUM-bank to halve copy count.
"""
import numpy as np

import concourse.bass as bass
import concourse.mybir as mybir
from concourse import bacc, tile
from concourse.bass_utils import run_bass_kernel_spmd

F16 = mybir.dt.float16
F32 = mybir.dt.float32
AF = mybir.ActivationFunctionType

NCORES = 8
B, T, N, DIN, U, O = 256, 64, 19, 100, 128, 100
BC = B // NCORES          # 32 batch per core
JT = 114                  # transpose window width (max tile rows)
J = 6                     # row tiles
R = 608                   # compact rows (no padding)
RV = BC * N               # valid rows = 608
HF = ((0, 304, 0, 3), (304, 304, 3, 6))  # half-chunks: (lo, sz, j0, j1)
RALL = 648   # transpose windows read [off, off+114) <= 646; zeros beyond 608
TOFF = (0, 114, 228, 304, 418, 532)      # compact tile offsets
TSZ = (114, 114, 76, 114, 114, 76)       # tile row counts
TB0 = (0, 6, 12, 16, 22, 28)             # first batch per tile
TNB = (6, 6, 4, 6, 6, 4)                 # batches per tile

CELLS = ("e0", "e1", "d0", "d1")
CELL_DIN = {"e0": DIN, "e1": U, "d0": DIN, "d1": U}


def _pack_layouts():
    p16, off = [], 0
    def add16(name, p, shape):
        nonlocal off
        n = int(np.prod(shape))
        p16.append((name, p, tuple(shape), off))
        off += n
    add16("bdcat3", JT, (J, 3 * JT))
    add16("bd12", JT, (J, 2 * JT))
    add16("ident", 128, (128,))
    add16("wp", U, (O,))
    add16("d0_ww", U, (3, 384))
    for c in CELLS:
        add16(f"{c}_wx", CELL_DIN[c], (3, 384))
        add16(f"{c}_wgh", U, (3, 256))
        add16(f"{c}_wch", U, (3, 128))
    f16_total = off
    p32, off = [], 0
    def add32(name, p, shape):
        nonlocal off
        n = int(np.prod(shape))
        p32.append((name, p, tuple(shape), off))
        off += n
    for c in CELLS:
        add32(f"{c}_bg", U, (2,))
        add32(f"{c}_bgn", U, (1,))
        add32(f"{c}_bc", U, (1,))
    add32("bp", O, (1,))
    return p16, f16_total, p32, off


PACK16, F16TOT, PACK32, F32TOT = _pack_layouts()


# --------------------------------------------------------------------------
# host-side weight preparation
# --------------------------------------------------------------------------

def _prep_host(inputs):
    f32 = np.float32
    S = np.asarray(inputs["support"], f32)
    P = [np.eye(N, dtype=f32), S, 2.0 * (S @ S) - np.eye(N, dtype=f32)]

    def bd_t(m, nb):
        Z = np.zeros((JT, JT), f32)
        for b in range(nb):
            Z[b * N:(b + 1) * N, b * N:(b + 1) * N] = P[m].T
        return Z

    vals = {}
    bdcat3 = np.zeros((JT, J, 3 * JT), f32)
    bd12 = np.zeros((JT, J, 2 * JT), f32)
    for j in range(J):
        nb = TNB[j]
        eye = np.zeros((JT, JT), f32)
        eye[:nb * N, :nb * N] = np.eye(nb * N, dtype=f32)
        bdcat3[:, j] = np.concatenate([eye, bd_t(1, nb), bd_t(2, nb)], axis=1)
        bd12[:, j] = np.concatenate([bd_t(1, nb), bd_t(2, nb)], axis=1)
    vals["bdcat3"] = bdcat3
    vals["bd12"] = bd12
    vals["ident"] = np.eye(128, dtype=f32)

    for c in CELLS:
        din = CELL_DIN[c]
        Wg = np.asarray(inputs[f"{c}_Wg"], f32)   # [(din+U)*3, 2U]
        Wc = np.asarray(inputs[f"{c}_Wc"], f32)   # [(din+U)*3, U]
        wx, wgh, wch = [], [], []
        for m in range(3):
            Wg_m, Wc_m = Wg[m::3], Wc[m::3]       # [(din+U), .]
            wx.append(np.concatenate([Wg_m[:din], Wc_m[:din]], axis=1))  # [din,384]
            wgh.append(Wg_m[din:])                # [U, 256]
            wch.append(Wc_m[din:])                # [U, 128]
        vals[f"{c}_wx"] = np.stack(wx, axis=1)    # [din, 3, 384]
        vals[f"{c}_wgh"] = np.stack(wgh, axis=1)  # [U, 3, 256]
        vals[f"{c}_wch"] = np.stack(wch, axis=1)  # [U, 3, 128]
        bg = np.asarray(inputs[f"{c}_bg"], f32)
        vals[f"{c}_bg"] = np.stack([bg[:U], bg[U:]], axis=1)  # [U, 2]
        vals[f"{c}_bgn"] = -bg[U:].reshape(U, 1)              # for 1-u = sig(-z)
        vals[f"{c}_bc"] = np.asarray(inputs[f"{c}_bc"], f32).reshape(U, 1)
    Wp = np.asarray(inputs["Wp"], f32)
    vals["wp"] = Wp                               # [U, O]
    vals["bp"] = np.asarray(inputs["bp"], f32).reshape(O, 1)
    # decoder projection-feedback fold: WW_m = Wp @ wx_m(d0)  [U, 384]
    vals["d0_ww"] = np.stack([Wp @ vals["d0_wx"][:, m] for m in range(3)], axis=1)
    assert np.abs(np.asarray(inputs["bp"], f32)).max() == 0.0, \
        "nonzero bp needs the rank-3 bias correction (not emitted)"

    pack16 = np.zeros((128, F16TOT), np.float16)
    for name, p, shape, off in PACK16:
        n = int(np.prod(shape))
        pack16[:p, off:off + n] = vals[name].reshape(p, n).astype(np.float16)
    pack32 = np.zeros((128, F32TOT), f32)
    for name, p, shape, off in PACK32:
        n = int(np.prod(shape))
        pack32[:p, off:off + n] = vals[name].reshape(p, n)
    return {"wpack16": pack16, "wpack32": pack32}


def _prep_xenc(enc, core, t_enc=T):
    """per-core encoder input -> [T, J, JT, DIN] fp16, zero row padding."""
    e = np.asarray(enc[core * BC:(core + 1) * BC, :t_enc], np.float32)
    e = e.transpose(1, 0, 2, 3)                                 # [T, BC, N, DIN]
    out = np.zeros((t_enc, J, JT, DIN), np.float16)
    for j in range(J):
        out[:, j, :TNB[j] * N] = \
            e[:, TB0[j]:TB0[j] + TNB[j]].reshape(t_enc, TNB[j] * N, DIN)
    return out


# --------------------------------------------------------------------------
# program builder
# --------------------------------------------------------------------------

def build_program(t_enc=T, t_dec=T):
    nc = bacc.Bacc()
    d = {}
    d["xenc"] = nc.dram_tensor("xenc", [t_enc, J, JT, DIN], F16, kind="ExternalInput")
    d["wpack16"] = nc.dram_tensor("wpack16", [128, F16TOT], F16, kind="ExternalInput")
    d["wpack32"] = nc.dram_tensor("wpack32", [128, F32TOT], F32, kind="ExternalInput")
    d["out"] = nc.dram_tensor("out", [t_dec, O, R], F32, kind="ExternalOutput")
    with tile.TileContext(nc) as tc:
        _emit(nc, tc, d, t_enc, t_dec)
    nc.finalize()
    return nc


def _emit(nc, tc, d, t_enc, t_dec):
    import contextlib
    stack = contextlib.ExitStack()
    with stack:
        perm = stack.enter_context(tc.tile_pool(name="perm", bufs=1))
        xpool = stack.enter_context(tc.tile_pool(name="xin", bufs=3))
        xcat = stack.enter_context(tc.tile_pool(name="xcat", bufs=3))
        m0pool = stack.enter_context(tc.tile_pool(name="m0p", bufs=4))
        s12p = stack.enter_context(tc.tile_pool(name="s12", bufs=3))
        hrm = stack.enter_context(tc.tile_pool(name="hrm", bufs=6))
        pg0 = stack.enter_context(tc.tile_pool(name="pg0", bufs=2, space="PSUM"))
        pg1 = stack.enter_context(tc.tile_pool(name="pg1", bufs=1, space="PSUM"))
        pcd = stack.enter_context(tc.tile_pool(name="pcd", bufs=1, space="PSUM"))
        SCR_BUFS = 4
        scr = stack.enter_context(tc.tile_pool(name="scr", bufs=SCR_BUFS, space="PSUM"))

        # ---- persistent weights ----
        wp16 = perm.tile([128, F16TOT], F16, name="wp16", tag="wp16")
        wp32 = perm.tile([128, F32TOT], F32, name="wp32", tag="wp32")
        nc.sync.dma_start(wp16[:], d["wpack16"][:])
        nc.sync.dma_start(wp32[:], d["wpack32"][:])
        w = {}
        for name, p, shape, off in PACK16:
            n = int(np.prod(shape))
            ap = wp16[:p, off:off + n]
            if len(shape) > 1:
                ap = ap.rearrange("p (a b) -> p a b", a=shape[0])
            w[name] = ap
        for name, p, shape, off in PACK32:
            n = int(np.prod(shape))
            w[name] = wp32[:p, off:off + n]

        # ---- persistent state (all fp16) ----
        h0seq = perm.tile([U, t_enc, RALL], F16, tag="h0seq")   # h0 after step t
        h1T = perm.tile([U, RALL], F16, tag="h1T")              # h1 state (enc+dec)
        hd0 = perm.tile([U, RALL], F16, tag="hd0")              # decoder h0 state
        hz16 = perm.tile([U, RALL], F16, tag="hz16")            # zeros
        zmix = perm.tile([U, 2, R], F16, tag="zmix")            # zero mix
        m1 = perm.tile([U, 2, R], F16, tag="m1")                # mix of h1 (P1,P2)
        m0d = perm.tile([U, 2, R], F16, tag="m0d")              # decoder mix of h0
        for tl in (h0seq, h1T, hd0, hz16, zmix, m1, m0d):
            nc.gpsimd.memset(tl[:], 0.0)

        # per-layer elementwise work tiles (a: layer0 cells, b: layer1 cells)
        wk = {}
        for sfx in ("a", "b"):
            wk[sfx] = {
                k: perm.tile([U, RALL if k == "rh" else R], F16,
                             tag=f"{k}{sfx}", name=f"{k}{sfx}")
                for k in ("rT", "uT", "cT", "tmp", "rh")
            }
            nc.gpsimd.memset(wk[sfx]["rh"][:], 0.0)
        LAYER_SFX = {"e0": "a", "d0": "a", "e1": "b", "d1": "b"}

        projT32 = perm.tile([O, R], F32, tag="projT32")
        tokT = perm.tile([1, 8], F16, tag="tokT")

        # ------------------------------------------------------------------
        # one-wait-per-matmul machinery (see v1 notes): pe_fence plants tiny
        # LDWEIGHTS wait-carriers; mm ties matmuls behind the latest fence.
        # ------------------------------------------------------------------
        from concourse.tile import add_dep_helper

        cur_fence = [None]

        def _raw(i):
            return i.ins if hasattr(i, "ins") and not isinstance(i.ins, list) else i

        def pe_fence(*tok_aps):
            for tok in tok_aps:
                if tok is None:
                    continue
                i = nc.tensor.ldweights(weights=tok)
                if cur_fence[0] is not None:
                    add_dep_helper(_raw(i), _raw(cur_fence[0]), sync=False)
                cur_fence[0] = i

        def mm(*args, after=None, **kw):
            i = nc.tensor.matmul(*args, **kw)
            if cur_fence[0] is not None:
                add_dep_helper(_raw(i), _raw(cur_fence[0]), sync=False)
            if after is not None:
                add_dep_helper(_raw(i), _raw(after), sync=False)
            return i

        def copy_act(dst, src):
            return nc.scalar.activation(dst, src, AF.Copy)

        def copy_dve(dst, src):
            return nc.vector.tensor_copy(dst, src)

        scr_state = {"k": 0, "toks": {}}

        def scr_alloc(shape):
            k = scr_state["k"]
            scr_state["k"] += 1
            tok = scr_state["toks"].get(k - SCR_BUFS)
            if tok is not None:
                pe_fence(tok)
            return scr.tile(shape, F32, tag="scr", name="scr"), k

        def scr_done(k, reader_instr, tok):
            scr_state["toks"][k] = tok
            return reader_instr

        # ------------------------------------------------------------------
        # node-mix of a feature-major fp16 tensor: src [U, RALL] -> dst [U,2,R]
        # (P1, P2 diffusion terms).  tr='pe': PE transpose (low latency);
        # tr='dma': DMA-xbar transpose (no PE/copy cost, ~us latency).
        # j-tiles are paired two-per-PSUM-bank to halve copy count.
        # ------------------------------------------------------------------
        def mix_state(src16, dst, eng, j0=0, j1=J, tr="pe"):
            cp = copy_act if eng == "act" else copy_dve
            cp_rm = copy_dve if eng in ("act", "dve2") else copy_act
            if eng == "dve2":
                cp = copy_dve
            pe_fence(src16[:1, TOFF[j0]:TOFF[j0] + 1])
            js = list(range(j0, j1))
            rms = {}
            for grp in [js[i:i + 3] for i in range(0, len(js), 3)]:
                ng = len(grp)
                rm = hrm.tile([128, 384], F16, tag="hrm", name="hrm")
                pt, k1 = scr_alloc([JT, 128 * ng])
                for gi, j in enumerate(grp):
                    mm(pt[:, gi * 128:gi * 128 + U],
                       src16[:, TOFF[j]:TOFF[j] + JT], w["ident"][:U, :U],
                       start=(gi == 0), stop=(gi == ng - 1))
                scr_done(k1, cp_rm(rm[:JT, :128 * ng], pt[:, :128 * ng]),
                         rm[:1, :1])
                for gi, j in enumerate(grp):
                    rms[j] = (rm, gi * 128)
            for grp in [js[i:i + 2] for i in range(0, len(js), 2)]:
                ng = len(grp)
                pm, k2 = scr_alloc([128, 228 * ng])
                for gi, j in enumerate(grp):
                    rm, off = rms[j]
                    mm(pm[:U, gi * 228:(gi + 1) * 228],
                       rm[:JT, off:off + U], w["bd12"][:, j],
                       start=(gi == 0), stop=(gi == ng - 1))
                last = None
                for gi, j in enumerate(grp):
                    last = cp(dst[:U, :, TOFF[j]:TOFF[j] + TSZ[j]],
                              pm[:U, gi * 228:(gi + 1) * 228]
                              .rearrange("d (m r) -> d m r", m=2)[:, :, :TSZ[j]])
                scr_done(k2, last, dst[:1, 0, TOFF[grp[-1]]:TOFF[grp[-1]] + 1])
            return dst

        # ------------------------------------------------------------------
        # x input staging for e0: row-major xtile -> mixed feature-major xc
        # ------------------------------------------------------------------
        xtiles = {}
        xcats = {}

        def dma_x(t):
            xt = xpool.tile([JT, J, DIN], F16, tag="xin", name="xin")
            nc.sync.dma_start(xt[:], d["xenc"][t].rearrange("j p q -> p j q"))
            xtiles[t] = xt

        def xstage_a(t):
            xc = xcat.tile([DIN, 3, R], F16, tag="xcat", name="xcat")
            xtile = xtiles.pop(t)
            pe_fence(xtile[:1, 0, :1])
            for j in range(J):
                pm, k = scr_alloc([128, 3 * JT])
                mm(pm[:DIN], xtile[:, j], w["bdcat3"][:, j])
                cp = copy_act if j % 2 == 0 else copy_dve
                scr_done(k, cp(xc[:, :, TOFF[j]:TOFF[j] + TSZ[j]],
                               pm[:DIN].rearrange("d (m r) -> d m r", m=3)
                               [:, :, :TSZ[j]]),
                         xc[:1, 0, TOFF[j]:TOFF[j] + 1])
            xcats[t] = xc

        # per-half, per-psum-group rolling ACT tokens: a bank may be
        # reclaimed once its ACT reader (sig-r / sig-u / tanh) has run.
        toks = {"r": [None, None], "u": [None, None], "c": [None, None]}
        TOKSLOT = {"r": 0, "u": 2, "c": 4}

        # ------------------------------------------------------------------
        # one DCGRU cell, split into phases so two cells can interleave
        # ------------------------------------------------------------------
        from types import SimpleNamespace

        def cell_mk(cn, xw, xrhs, hsrc, hmix, hdst):
            """xw/xrhs: x-side weight name + rhs fn (None to skip, go-symbol=0).
            hsrc [U,RALL] fp16 prev state; hmix [U,2,R] its P1/P2 mix;
            hdst [U,RALL] fp16 new state (may alias hsrc)."""
            sfx = LAYER_SFX[cn]
            rT, uT, cT, tmp = (
                wk[sfx][k] for k in ("rT", "uT", "cT", "tmp"))
            rh16 = wk[sfx]["rh"]
            st = {"gps": {}, "lastw": {}, "s12rh": None}

            def gmm(hh, c, out, lhsT, rhs, is_last=False):
                key = (hh, c)
                i = mm(out, lhsT, rhs, start=(key not in st["lastw"]),
                       stop=is_last, after=st["lastw"].get(key))
                st["lastw"][key] = i
                return i

            def settok(kind, hh, src_ap):
                # fence target: the ACT instruction's own output slice -- a
                # pe_fence ldweights reading it waits for the activation, which
                # implies its PSUM-bank read is complete.  No token copy.
                toks[kind][hh] = src_ap

            def p_gates_h():
                for hh, (lo, sz, j0, j1) in enumerate(HF):
                    pe_fence(toks["r"][hh])
                    gp0 = pg0.tile([U, 304], F32, tag="gp0", name="gp0")
                    st["gps"][hh] = [gp0, None, None]
                    for m in range(3):
                        rhs = hsrc[:, lo:lo + sz] if m == 0 else hmix[:, m - 1, lo:lo + sz]
                        gmm(hh, 0, gp0[:, :sz], w[f"{cn}_wgh"][:, m, 0:128], rhs,
                            is_last=(m == 2 and xrhs is None))
                for hh, (lo, sz, j0, j1) in enumerate(HF):
                    pe_fence(toks["u"][0], toks["u"][1])
                    gp1 = pg1.tile([U, 304], F32, tag="gp1", name="gp1")
                    st["gps"][hh][1] = gp1
                    for m in range(3):
                        rhs = hsrc[:, lo:lo + sz] if m == 0 else hmix[:, m - 1, lo:lo + sz]
                        gmm(hh, 1, gp1[:, :sz], w[f"{cn}_wgh"][:, m, 128:256], rhs,
                            is_last=(m == 2 and xrhs is None))

            def p_gates_x():
                if xrhs is None:
                    return
                for hh, (lo, sz, j0, j1) in enumerate(HF):
                    for m in range(3):
                        gmm(hh, 0, st["gps"][hh][0][:, :sz], w[xw][:, m, 0:128],
                            xrhs(m)[:, lo:lo + sz], is_last=(m == 2))
                for hh, (lo, sz, j0, j1) in enumerate(HF):
                    for m in range(3):
                        gmm(hh, 1, st["gps"][hh][1][:, :sz], w[xw][:, m, 128:256],
                            xrhs(m)[:, lo:lo + sz], is_last=(m == 2))

            def p_sig_rh():
                s12rh = s12p.tile([U, 2, R], F16, tag=f"s12rh{sfx}", name="s12rh")
                st["s12rh"] = s12rh
                for hh, (lo, sz, j0, j1) in enumerate(HF):
                    nc.scalar.activation(rT[:, lo:lo + sz], st["gps"][hh][0][:, :sz],
                                         AF.Sigmoid, bias=w[f"{cn}_bg"][:, 0:1])
                    settok("r", hh, rT[:1, lo:lo + 1])
                    nc.scalar.activation(uT[:, lo:lo + sz], st["gps"][hh][1][:, :sz],
                                         AF.Sigmoid, bias=w[f"{cn}_bg"][:, 1:2])
                    settok("u", hh, uT[:1, lo:lo + 1])
                    nc.vector.tensor_mul(rh16[:, lo:lo + sz], rT[:, lo:lo + sz],
                                         hsrc[:, lo:lo + sz])
                    mix_state(rh16, s12rh, "dve2", j0, j1, tr="pe")

            def p_cand():
                s12rh = st["s12rh"]
                for hh, (lo, sz, j0, j1) in enumerate(HF):
                    pe_fence(toks["c"][0], toks["c"][1])
                    cpb = pcd.tile([U, 304], F32, tag="cp", name="cp")
                    st["gps"][hh][2] = cpb
                    if xrhs is not None:
                        for m in range(3):
                            gmm(hh, 2, cpb[:, :sz], w[xw][:, m, 256:384],
                                xrhs(m)[:, lo:lo + sz])
                    for m in range(3):
                        rhs = rh16[:, lo:lo + sz] if m == 0 else s12rh[:, m - 1, lo:lo + sz]
                        gmm(hh, 2, cpb[:, :sz], w[f"{cn}_wch"][:, m], rhs,
                            is_last=(m == 2))

            def p_tail_h(hh):
                for hh, (lo, sz, j0, j1) in [(hh, HF[hh])]:
                    sl = slice(lo, lo + sz)
                    nc.scalar.activation(cT[:, sl], st["gps"][hh][2][:, :sz],
                                         AF.Tanh, bias=w[f"{cn}_bc"][:, 0:1])
                    settok("c", hh, cT[:1, lo:lo + 1])
                    nc.vector.tensor_sub(tmp[:, sl], hsrc[:, sl], cT[:, sl])
                    nc.vector.tensor_mul(tmp[:, sl], uT[:, sl], tmp[:, sl])
                    nc.vector.tensor_add(hdst[:, sl], cT[:, sl], tmp[:, sl])

            def p_tail():
                p_tail_h(0)
                p_tail_h(1)

            return SimpleNamespace(p_gates_h=p_gates_h, p_gates_x=p_gates_x,
                                   p_sig_rh=p_sig_rh, p_cand=p_cand,
                                   p_tail=p_tail, p_tail_h=p_tail_h)

        def cell(cn, xw, xrhs, hsrc, hmix, hdst):
            c = cell_mk(cn, xw, xrhs, hsrc, hmix, hdst)
            c.p_gates_h(); c.p_gates_x(); c.p_sig_rh(); c.p_cand(); c.p_tail()

        # ------------------------------------------------------------------
        # encoder: e1 runs one step behind e0, phase-interleaved
        # ------------------------------------------------------------------
        m0_tiles = {}

        def mk_mix0(t, tr):
            mt = m0pool.tile([U, 2, R], F16, tag="m0", name="m0")
            mix_state(h0seq[:, t], mt, "act", 0, J, tr=tr)
            m0_tiles[t] = mt

        def e1_mk(tt):
            return cell_mk("e1", "e1_wx",
                           lambda m, tt=tt: h0seq[:, tt] if m == 0
                           else m0_tiles[tt][:, m - 1],
                           hsrc=(hz16 if tt == 0 else h1T), hmix=m1, hdst=h1T)

        def e0_mk(t):
            xc = xcats.pop(t)
            return cell_mk("e0", "e0_wx", lambda m, xc=xc: xc[:, m],
                           hsrc=(hz16 if t == 0 else h0seq[:, t - 1]),
                           hmix=(zmix if t == 0 else m0_tiles[t - 1]),
                           hdst=h0seq[:, t])

        dma_x(0)
        if t_enc > 1:
            dma_x(1)
        xstage_a(0)
        A = e0_mk(0)
        A.p_gates_h(); A.p_gates_x()
        for t in range(t_enc):
            if t + 2 < t_enc:
                dma_x(t + 2)
            if t + 1 < t_enc:
                xstage_a(t + 1)
            A.p_sig_rh()
            B = e1_mk(t - 1) if t >= 1 else None
            if B:
                B.p_gates_h(); B.p_gates_x()
            A.p_cand()
            if B:
                B.p_sig_rh()
            A.p_tail_h(0)
            if B:
                B.p_cand()
            mt = m0pool.tile([U, 2, R], F16, tag="m0", name="m0")
            m0_tiles[t] = mt
            mix_state(h0seq[:, t], mt, "act", 0, 3, tr="pe")
            A.p_tail_h(1)
            mix_state(h0seq[:, t], mt, "act", 3, 6, tr="pe")
            # software pipeline: next e0's h-side gate matmuls fill the
            # e1-tail / mix1 stall window (their inputs are ready here)
            if t + 1 < t_enc:
                An = e0_mk(t + 1)
                An.p_gates_h(); An.p_gates_x()
            if B:
                B.p_tail()
                mix_state(h1T, m1, "act", 0, J, tr="pe")
            if t + 1 < t_enc:
                A = An
        B = e1_mk(t_enc - 1)
        B.p_gates_h(); B.p_gates_x(); B.p_sig_rh(); B.p_cand(); B.p_tail()
        mix_state(h1T, m1, "act", 0, J, tr="pe")

        # ------------------------------------------------------------------
        # decoder: d0 x-side folded through WW on (h1, m1); proj output-only.
        # d1's h-side gate matmuls hoisted into d0's stall window.
        # ------------------------------------------------------------------
        def d0_mk(s):
            if s == 0:
                return cell_mk("d0", None, None,
                               hsrc=h0seq[:, t_enc - 1],
                               hmix=m0_tiles[t_enc - 1], hdst=hd0)
            return cell_mk("d0", "d0_ww",
                           lambda m: h1T if m == 0 else m1[:, m - 1],
                           hsrc=hd0, hmix=m0d, hdst=hd0)

        D0 = d0_mk(0)
        D0.p_gates_h()
        for s in range(t_dec):
            D1 = cell_mk("d1", "d1_wx",
                         lambda m: hd0 if m == 0 else m0d[:, m - 1],
                         hsrc=h1T, hmix=m1, hdst=h1T)
            D0.p_gates_x(); D0.p_sig_rh()
            D1.p_gates_h()
            D0.p_cand()
            D0.p_tail_h(0)
            mix_state(hd0, m0d, "dve", 0, 3, tr="pe")
            D0.p_tail_h(1)
            mix_state(hd0, m0d, "dve", 3, 6, tr="pe")
            D1.p_gates_x(); D1.p_sig_rh()
            # software pipeline: next d0's h-side gate matmuls (hd0_s and
            # m0d_s are ready) fill d1's cand/tail stall window
            if s + 1 < t_dec:
                D0n = d0_mk(s + 1)
                D0n.p_gates_h()
            D1.p_cand(); D1.p_tail()
            if s + 1 < t_dec:
                D0 = D0n
            # projection (output only, off critical path)
            pe_fence(h1T[:1, :1])
            for lo, sz, j0, j1 in HF:
                pp, k = scr_alloc([O, 304])
                mm(pp[:, :sz], w["wp"][:], h1T[:, lo:lo + sz])
                nc.scalar.activation(projT32[:, lo:lo + sz], pp[:, :sz],
                                     AF.Copy)
                scr_done(k, copy_act(tokT[:1, 6:7], projT32[:1, lo:lo + 1]),
                         tokT[:1, 6:7])
            nc.vector.tensor_scalar_add(projT32[:], projT32[:], w["bp"][:, 0:1])
            nc.sync.dma_start(d["out"][s], projT32[:])
            mix_state(h1T, m1, "act", 0, J, tr="pe")



# --------------------------------------------------------------------------
# entry point
# --------------------------------------------------------------------------

def kernel(**inputs):
    arrs = _prep_host(inputs)
    nc = build_program(T, T)
    in_maps = []
    for core in range(NCORES):
        m = dict(arrs)
        m["xenc"] = _prep_xenc(inputs["encoder_inputs"], core)
        in_maps.append(m)
    res = run_bass_kernel_spmd(nc, in_maps, list(range(NCORES))).results
    outs = []
    for core in range(NCORES):
        o = np.asarray(res[core]["out"], np.float32)[:, :, :RV]   # [T, O, RV]
        o = o.reshape(T, O, BC, N).transpose(2, 0, 3, 1)          # [BC, T, N, O]
        outs.append(o)
    return np.ascontiguousarray(np.concatenate(outs, axis=0))



# revision 8
# speedup vs baseline: 1.0714x; 1.0089x over previous
"""DCRNN (nn_DCRNNModel_nextTimePred) Bass/Tile kernel for 8 TRN2 NeuronCores, v2.

Sharding: data-parallel over batch (B=256 -> 32 per core), weights replicated.
Layout: activations feature-major [feat<=128, R] with R = 32*19 = 608 rows
compact 608 rows in 6 tiles [114,114,76|114,114,76] aligned to 304-halves.

v2 structural changes vs v1:
- fp16 state + fp16 elementwise GRU tail (DVE 2x mode), no separate casts.
- Each layer's node-mix (BD-matmul of its fresh h) is computed ONCE per step
  and shared by (a) that layer's own gates next step and (b) the next layer's
  x-side this step.  m=0 term is the state tensor itself.
- Decoder projection feedback folded into the weights: (P_k(h1 Wp))Wx_k =
  (P_k h1)(Wp Wx_k), so d0's x-side consumes the already-computed mix of h1
  with combined weights WW_k = Wp @ Wx_k.  proj is computed only for output,
  off the critical path.  (bp = 0 for this problem so no bias correction.)
- Gate/cand matmuls at N=304 halves (compact rows, no padding).
- State mixes use DMA transposes (cross-step latency tolerant); the rh mix and
  the decoder's h0-mix (same-step consumers) use PE transposes.  Transpose and
  mix outputs are paired two-j-tiles-per-PSUM-bank to halve copy count.
"""
import numpy as np

import concourse.bass as bass
import concourse.mybir as mybir
from concourse import bacc, tile
from concourse.bass_utils import run_bass_kernel_spmd

F16 = mybir.dt.float16
F32 = mybir.dt.float32
AF = mybir.ActivationFunctionType

NCORES = 8
B, T, N, DIN, U, O = 256, 64, 19, 100, 128, 100
BC = B // NCORES          # 32 batch per core
JT = 114                  # transpose window width (max tile rows)
J = 6                     # row tiles
R = 608                   # compact rows (no padding)
RV = BC * N               # valid rows = 608
HF = ((0, 304, 0, 3), (304, 304, 3, 6))  # half-chunks: (lo, sz, j0, j1)
RALL = 648   # transpose windows read [off, off+114) <= 646; zeros beyond 608
TOFF = (0, 114, 228, 304, 418, 532)      # compact tile offsets
TSZ = (114, 114, 76, 114, 114, 76)       # tile row counts
TB0 = (0, 6, 12, 16, 22, 28)             # first batch per tile
TNB = (6, 6, 4, 6, 6, 4)                 # batches per tile

CELLS = ("e0", "e1", "d0", "d1")
CELL_DIN = {"e0": DIN, "e1": U, "d0": DIN, "d1": U}


def _pack_layouts():
    p16, off = [], 0
    def add16(name, p, shape):
        nonlocal off
        n = int(np.prod(shape))
        p16.append((name, p, tuple(shape), off))
        off += n
    add16("bdcat3", JT, (J, 3 * JT))
    add16("bd12", JT, (J, 2 * JT))
    add16("ident", 128, (128,))
    add16("wp", U, (O,))
    add16("d0_ww", U, (3, 384))
    for c in CELLS:
        add16(f"{c}_wx", CELL_DIN[c], (3, 384))
        add16(f"{c}_wgh", U, (3, 256))
        add16(f"{c}_wch", U, (3, 128))
    f16_total = off
    p32, off = [], 0
    def add32(name, p, shape):
        nonlocal off
        n = int(np.prod(shape))
        p32.append((name, p, tuple(shape), off))
        off += n
    for c in CELLS:
        add32(f"{c}_bg", U, (2,))
        add32(f"{c}_bgn", U, (1,))
        add32(f"{c}_bc", U, (1,))
    add32("bp", O, (1,))
    return p16, f16_total, p32, off


PACK16, F16TOT, PACK32, F32TOT = _pack_layouts()


# --------------------------------------------------------------------------
# host-side weight preparation
# --------------------------------------------------------------------------

def _prep_host(inputs):
    f32 = np.float32
    S = np.asarray(inputs["support"], f32)
    P = [np.eye(N, dtype=f32), S, 2.0 * (S @ S) - np.eye(N, dtype=f32)]

    def bd_t(m, nb):
        Z = np.zeros((JT, JT), f32)
        for b in range(nb):
            Z[b * N:(b + 1) * N, b * N:(b + 1) * N] = P[m].T
        return Z

    vals = {}
    bdcat3 = np.zeros((JT, J, 3 * JT), f32)
    bd12 = np.zeros((JT, J, 2 * JT), f32)
    for j in range(J):
        nb = TNB[j]
        eye = np.zeros((JT, JT), f32)
        eye[:nb * N, :nb * N] = np.eye(nb * N, dtype=f32)
        bdcat3[:, j] = np.concatenate([eye, bd_t(1, nb), bd_t(2, nb)], axis=1)
        bd12[:, j] = np.concatenate([bd_t(1, nb), bd_t(2, nb)], axis=1)
    vals["bdcat3"] = bdcat3
    vals["bd12"] = bd12
    vals["ident"] = np.eye(128, dtype=f32)

    for c in CELLS:
        din = CELL_DIN[c]
        Wg = np.asarray(inputs[f"{c}_Wg"], f32)   # [(din+U)*3, 2U]
        Wc = np.asarray(inputs[f"{c}_Wc"], f32)   # [(din+U)*3, U]
        wx, wgh, wch = [], [], []
        for m in range(3):
            Wg_m, Wc_m = Wg[m::3], Wc[m::3]       # [(din+U), .]
            wx.append(np.concatenate([Wg_m[:din], Wc_m[:din]], axis=1))  # [din,384]
            wgh.append(Wg_m[din:])                # [U, 256]
            wch.append(Wc_m[din:])                # [U, 128]
        vals[f"{c}_wx"] = np.stack(wx, axis=1)    # [din, 3, 384]
        vals[f"{c}_wgh"] = np.stack(wgh, axis=1)  # [U, 3, 256]
        vals[f"{c}_wch"] = np.stack(wch, axis=1)  # [U, 3, 128]
        bg = np.asarray(inputs[f"{c}_bg"], f32)
        vals[f"{c}_bg"] = np.stack([bg[:U], bg[U:]], axis=1)  # [U, 2]
        vals[f"{c}_bgn"] = -bg[U:].reshape(U, 1)              # for 1-u = sig(-z)
        vals[f"{c}_bc"] = np.asarray(inputs[f"{c}_bc"], f32).reshape(U, 1)
    Wp = np.asarray(inputs["Wp"], f32)
    vals["wp"] = Wp                               # [U, O]
    vals["bp"] = np.asarray(inputs["bp"], f32).reshape(O, 1)
    # decoder projection-feedback fold: WW_m = Wp @ wx_m(d0)  [U, 384]
    vals["d0_ww"] = np.stack([Wp @ vals["d0_wx"][:, m] for m in range(3)], axis=1)
    assert np.abs(np.asarray(inputs["bp"], f32)).max() == 0.0, \
        "nonzero bp needs the rank-3 bias correction (not emitted)"

    pack16 = np.zeros((128, F16TOT), np.float16)
    for name, p, shape, off in PACK16:
        n = int(np.prod(shape))
        pack16[:p, off:off + n] = vals[name].reshape(p, n).astype(np.float16)
    pack32 = np.zeros((128, F32TOT), f32)
    for name, p, shape, off in PACK32:
        n = int(np.prod(shape))
        pack32[:p, off:off + n] = vals[name].reshape(p, n)
    return {"wpack16": pack16, "wpack32": pack32}


def _prep_xenc(enc, core, t_enc=T):
    """per-core encoder input -> [T, J, JT, DIN] fp16, zero row padding."""
    e = np.asarray(enc[core * BC:(core + 1) * BC, :t_enc], np.float32)
    e = e.transpose(1, 0, 2, 3)                                 # [T, BC, N, DIN]
    out = np.zeros((t_enc, J, JT, DIN), np.float16)
    for j in range(J):
        out[:, j, :TNB[j] * N] = \
            e[:, TB0[j]:TB0[j] + TNB[j]].reshape(t_enc, TNB[j] * N, DIN)
    return out


# --------------------------------------------------------------------------
# program builder
# --------------------------------------------------------------------------

def build_program(t_enc=T, t_dec=T):
    nc = bacc.Bacc()
    d = {}
    d["xenc"] = nc.dram_tensor("xenc", [t_enc, J, JT, DIN], F16, kind="ExternalInput")
    d["wpack16"] = nc.dram_tensor("wpack16", [128, F16TOT], F16, kind="ExternalInput")
    d["wpack32"] = nc.dram_tensor("wpack32", [128, F32TOT], F32, kind="ExternalInput")
    d["out"] = nc.dram_tensor("out", [t_dec, O, R], F32, kind="ExternalOutput")
    with tile.TileContext(nc) as tc:
        _emit(nc, tc, d, t_enc, t_dec)
    nc.finalize()
    return nc


def _emit(nc, tc, d, t_enc, t_dec):
    import contextlib
    stack = contextlib.ExitStack()
    with stack:
        perm = stack.enter_context(tc.tile_pool(name="perm", bufs=1))
        xpool = stack.enter_context(tc.tile_pool(name="xin", bufs=3))
        xcat = stack.enter_context(tc.tile_pool(name="xcat", bufs=3))
        m0pool = stack.enter_context(tc.tile_pool(name="m0p", bufs=4))
        s12p = stack.enter_context(tc.tile_pool(name="s12", bufs=3))
        hrm = stack.enter_context(tc.tile_pool(name="hrm", bufs=6))
        pg0 = stack.enter_context(tc.tile_pool(name="pg0", bufs=2, space="PSUM"))
        pg1 = stack.enter_context(tc.tile_pool(name="pg1", bufs=1, space="PSUM"))
        pcd = stack.enter_context(tc.tile_pool(name="pcd", bufs=1, space="PSUM"))
        SCR_BUFS = 4
        scr = stack.enter_context(tc.tile_pool(name="scr", bufs=SCR_BUFS, space="PSUM"))

        # ---- persistent weights ----
        wp16 = perm.tile([128, F16TOT], F16, name="wp16", tag="wp16")
        wp32 = perm.tile([128, F32TOT], F32, name="wp32", tag="wp32")
        nc.sync.dma_start(wp16[:], d["wpack16"][:])
        nc.sync.dma_start(wp32[:], d["wpack32"][:])
        w = {}
        for name, p, shape, off in PACK16:
            n = int(np.prod(shape))
            ap = wp16[:p, off:off + n]
            if len(shape) > 1:
                ap = ap.rearrange("p (a b) -> p a b", a=shape[0])
            w[name] = ap
        for name, p, shape, off in PACK32:
            n = int(np.prod(shape))
            w[name] = wp32[:p, off:off + n]

        # ---- persistent state (all fp16) ----
        h0seq = perm.tile([U, t_enc, RALL], F16, tag="h0seq")   # h0 after step t
        h1T = perm.tile([U, RALL], F16, tag="h1T")              # h1 state (enc+dec)
        hd0 = perm.tile([U, RALL], F16, tag="hd0")              # decoder h0 state
        hz16 = perm.tile([U, RALL], F16, tag="hz16")            # zeros
        zmix = perm.tile([U, 2, R], F16, tag="zmix")            # zero mix
        m1 = perm.tile([U, 2, R], F16, tag="m1")                # mix of h1 (P1,P2)
        m0d = perm.tile([U, 2, R], F16, tag="m0d")              # decoder mix of h0
        for tl in (h0seq, h1T, hd0, hz16, zmix, m1, m0d):
            nc.gpsimd.memset(tl[:], 0.0)

        # per-layer elementwise work tiles (a: layer0 cells, b: layer1 cells)
        wk = {}
        for sfx in ("a", "b"):
            wk[sfx] = {
                k: perm.tile([U, RALL if k == "rh" else R], F16,
                             tag=f"{k}{sfx}", name=f"{k}{sfx}")
                for k in ("rT", "uT", "cT", "tmp", "rh")
            }
            nc.gpsimd.memset(wk[sfx]["rh"][:], 0.0)
        LAYER_SFX = {"e0": "a", "d0": "a", "e1": "b", "d1": "b"}

        projT32 = perm.tile([O, R], F32, tag="projT32")
        tokT = perm.tile([1, 8], F16, tag="tokT")

        # ------------------------------------------------------------------
        # one-wait-per-matmul machinery (see v1 notes): pe_fence plants tiny
        # LDWEIGHTS wait-carriers; mm ties matmuls behind the latest fence.
        # ------------------------------------------------------------------
        from concourse.tile import add_dep_helper

        cur_fence = [None]

        def _raw(i):
            return i.ins if hasattr(i, "ins") and not isinstance(i.ins, list) else i

        def pe_fence(*tok_aps):
            for tok in tok_aps:
                if tok is None:
                    continue
                i = nc.tensor.ldweights(weights=tok)
                if cur_fence[0] is not None:
                    add_dep_helper(_raw(i), _raw(cur_fence[0]), sync=False)
                cur_fence[0] = i

        def mm(*args, after=None, **kw):
            i = nc.tensor.matmul(*args, **kw)
            if cur_fence[0] is not None:
                add_dep_helper(_raw(i), _raw(cur_fence[0]), sync=False)
            if after is not None:
                add_dep_helper(_raw(i), _raw(after), sync=False)
            return i

        def copy_act(dst, src):
            return nc.scalar.activation(dst, src, AF.Copy)

        def copy_dve(dst, src):
            return nc.vector.tensor_copy(dst, src)

        scr_state = {"k": 0, "toks": {}}

        def scr_alloc(shape, dtype=F32):
            k = scr_state["k"]
            scr_state["k"] += 1
            tok = scr_state["toks"].get(k - SCR_BUFS)
            if tok is not None:
                pe_fence(tok)
            return scr.tile(shape, dtype, tag="scr", name="scr"), k

        def scr_done(k, reader_instr, tok):
            scr_state["toks"][k] = tok
            return reader_instr

        # ------------------------------------------------------------------
        # node-mix of a feature-major fp16 tensor: src [U, RALL] -> dst [U,2,R]
        # (P1, P2 diffusion terms).  tr='pe': PE transpose (low latency);
        # tr='dma': DMA-xbar transpose (no PE/copy cost, ~us latency).
        # j-tiles are paired two-per-PSUM-bank to halve copy count.
        # ------------------------------------------------------------------
        def mix_state(src16, dst, eng, j0=0, j1=J, tr="pe"):
            cp = copy_act if eng == "act" else copy_dve
            cp_rm = copy_dve if eng in ("act", "dve2") else copy_act
            if eng == "dve2":
                cp = copy_dve
            pe_fence(src16[:1, TOFF[j0]:TOFF[j0] + 1])
            js = list(range(j0, j1))
            rms = {}
            for grp in [js[i:i + 3] for i in range(0, len(js), 3)]:
                ng = len(grp)
                rm = hrm.tile([128, 384], F16, tag="hrm", name="hrm")
                pt, k1 = scr_alloc([JT, 128 * ng], F16)
                for gi, j in enumerate(grp):
                    mm(pt[:, gi * 128:gi * 128 + U],
                       src16[:, TOFF[j]:TOFF[j] + JT], w["ident"][:U, :U],
                       start=(gi == 0), stop=(gi == ng - 1),
                       is_transpose=True)
                scr_done(k1, cp_rm(rm[:JT, :128 * ng], pt[:, :128 * ng]),
                         rm[:1, :1])
                for gi, j in enumerate(grp):
                    rms[j] = (rm, gi * 128)
            for grp in [js[i:i + 2] for i in range(0, len(js), 2)]:
                ng = len(grp)
                pm, k2 = scr_alloc([128, 228 * ng])
                for gi, j in enumerate(grp):
                    rm, off = rms[j]
                    mm(pm[:U, gi * 228:(gi + 1) * 228],
                       rm[:JT, off:off + U], w["bd12"][:, j],
                       start=(gi == 0), stop=(gi == ng - 1))
                last = None
                for gi, j in enumerate(grp):
                    last = cp(dst[:U, :, TOFF[j]:TOFF[j] + TSZ[j]],
                              pm[:U, gi * 228:(gi + 1) * 228]
                              .rearrange("d (m r) -> d m r", m=2)[:, :, :TSZ[j]])
                scr_done(k2, last, dst[:1, 0, TOFF[grp[-1]]:TOFF[grp[-1]] + 1])
            return dst

        # ------------------------------------------------------------------
        # x input staging for e0: row-major xtile -> mixed feature-major xc
        # ------------------------------------------------------------------
        xtiles = {}
        xcats = {}

        def dma_x(t):
            xt = xpool.tile([JT, J, DIN], F16, tag="xin", name="xin")
            nc.sync.dma_start(xt[:], d["xenc"][t].rearrange("j p q -> p j q"))
            xtiles[t] = xt

        def xstage_a(t):
            xc = xcat.tile([DIN, 3, R], F16, tag="xcat", name="xcat")
            xtile = xtiles.pop(t)
            pe_fence(xtile[:1, 0, :1])
            for j in range(J):
                pm, k = scr_alloc([128, 3 * JT])
                mm(pm[:DIN], xtile[:, j], w["bdcat3"][:, j])
                cp = copy_act if j % 2 == 0 else copy_dve
                scr_done(k, cp(xc[:, :, TOFF[j]:TOFF[j] + TSZ[j]],
                               pm[:DIN].rearrange("d (m r) -> d m r", m=3)
                               [:, :, :TSZ[j]]),
                         xc[:1, 0, TOFF[j]:TOFF[j] + 1])
            xcats[t] = xc

        # per-half, per-psum-group rolling ACT tokens: a bank may be
        # reclaimed once its ACT reader (sig-r / sig-u / tanh) has run.
        toks = {"r": [None, None], "u": [None, None], "c": [None, None]}
        TOKSLOT = {"r": 0, "u": 2, "c": 4}

        # ------------------------------------------------------------------
        # one DCGRU cell, split into phases so two cells can interleave
        # ------------------------------------------------------------------
        from types import SimpleNamespace

        def cell_mk(cn, xw, xrhs, hsrc, hmix, hdst):
            """xw/xrhs: x-side weight name + rhs fn (None to skip, go-symbol=0).
            hsrc [U,RALL] fp16 prev state; hmix [U,2,R] its P1/P2 mix;
            hdst [U,RALL] fp16 new state (may alias hsrc)."""
            sfx = LAYER_SFX[cn]
            rT, uT, cT, tmp = (
                wk[sfx][k] for k in ("rT", "uT", "cT", "tmp"))
            rh16 = wk[sfx]["rh"]
            st = {"gps": {}, "lastw": {}, "s12rh": None}

            def gmm(hh, c, out, lhsT, rhs, is_last=False):
                key = (hh, c)
                i = mm(out, lhsT, rhs, start=(key not in st["lastw"]),
                       stop=is_last, after=st["lastw"].get(key))
                st["lastw"][key] = i
                return i

            def settok(kind, hh, src_ap):
                # fence target: the ACT instruction's own output slice -- a
                # pe_fence ldweights reading it waits for the activation, which
                # implies its PSUM-bank read is complete.  No token copy.
                toks[kind][hh] = src_ap

            def p_gates_h():
                for hh, (lo, sz, j0, j1) in enumerate(HF):
                    pe_fence(toks["r"][hh])
                    gp0 = pg0.tile([U, 304], F32, tag="gp0", name="gp0")
                    st["gps"][hh] = [gp0, None, None]
                    for m in range(3):
                        rhs = hsrc[:, lo:lo + sz] if m == 0 else hmix[:, m - 1, lo:lo + sz]
                        gmm(hh, 0, gp0[:, :sz], w[f"{cn}_wgh"][:, m, 0:128], rhs,
                            is_last=(m == 2 and xrhs is None))
                for hh, (lo, sz, j0, j1) in enumerate(HF):
                    pe_fence(toks["u"][0], toks["u"][1])
                    gp1 = pg1.tile([U, 304], F32, tag="gp1", name="gp1")
                    st["gps"][hh][1] = gp1
                    for m in range(3):
                        rhs = hsrc[:, lo:lo + sz] if m == 0 else hmix[:, m - 1, lo:lo + sz]
                        gmm(hh, 1, gp1[:, :sz], w[f"{cn}_wgh"][:, m, 128:256], rhs,
                            is_last=(m == 2 and xrhs is None))

            def p_gates_x():
                if xrhs is None:
                    return
                for hh, (lo, sz, j0, j1) in enumerate(HF):
                    for m in range(3):
                        gmm(hh, 0, st["gps"][hh][0][:, :sz], w[xw][:, m, 0:128],
                            xrhs(m)[:, lo:lo + sz], is_last=(m == 2))
                for hh, (lo, sz, j0, j1) in enumerate(HF):
                    for m in range(3):
                        gmm(hh, 1, st["gps"][hh][1][:, :sz], w[xw][:, m, 128:256],
                            xrhs(m)[:, lo:lo + sz], is_last=(m == 2))

            def p_sig_rh():
                s12rh = s12p.tile([U, 2, R], F16, tag=f"s12rh{sfx}", name="s12rh")
                st["s12rh"] = s12rh
                for hh, (lo, sz, j0, j1) in enumerate(HF):
                    nc.scalar.activation(rT[:, lo:lo + sz], st["gps"][hh][0][:, :sz],
                                         AF.Sigmoid, bias=w[f"{cn}_bg"][:, 0:1])
                    settok("r", hh, rT[:1, lo:lo + 1])
                    nc.scalar.activation(uT[:, lo:lo + sz], st["gps"][hh][1][:, :sz],
                                         AF.Sigmoid, bias=w[f"{cn}_bg"][:, 1:2])
                    settok("u", hh, uT[:1, lo:lo + 1])
                    nc.vector.tensor_mul(rh16[:, lo:lo + sz], rT[:, lo:lo + sz],
                                         hsrc[:, lo:lo + sz])
                    mix_state(rh16, s12rh, "dve2", j0, j1, tr="pe")

            def p_cand():
                s12rh = st["s12rh"]
                for hh, (lo, sz, j0, j1) in enumerate(HF):
                    pe_fence(toks["c"][0], toks["c"][1])
                    cpb = pcd.tile([U, 304], F32, tag="cp", name="cp")
                    st["gps"][hh][2] = cpb
                    if xrhs is not None:
                        for m in range(3):
                            gmm(hh, 2, cpb[:, :sz], w[xw][:, m, 256:384],
                                xrhs(m)[:, lo:lo + sz])
                    for m in range(3):
                        rhs = rh16[:, lo:lo + sz] if m == 0 else s12rh[:, m - 1, lo:lo + sz]
                        gmm(hh, 2, cpb[:, :sz], w[f"{cn}_wch"][:, m], rhs,
                            is_last=(m == 2))

            def p_tail_h(hh):
                for hh, (lo, sz, j0, j1) in [(hh, HF[hh])]:
                    sl = slice(lo, lo + sz)
                    nc.scalar.activation(cT[:, sl], st["gps"][hh][2][:, :sz],
                                         AF.Tanh, bias=w[f"{cn}_bc"][:, 0:1])
                    settok("c", hh, cT[:1, lo:lo + 1])
                    nc.vector.tensor_sub(tmp[:, sl], hsrc[:, sl], cT[:, sl])
                    nc.vector.tensor_mul(tmp[:, sl], uT[:, sl], tmp[:, sl])
                    nc.vector.tensor_add(hdst[:, sl], cT[:, sl], tmp[:, sl])

            def p_tail():
                p_tail_h(0)
                p_tail_h(1)

            return SimpleNamespace(p_gates_h=p_gates_h, p_gates_x=p_gates_x,
                                   p_sig_rh=p_sig_rh, p_cand=p_cand,
                                   p_tail=p_tail, p_tail_h=p_tail_h)

        def cell(cn, xw, xrhs, hsrc, hmix, hdst):
            c = cell_mk(cn, xw, xrhs, hsrc, hmix, hdst)
            c.p_gates_h(); c.p_gates_x(); c.p_sig_rh(); c.p_cand(); c.p_tail()

        # ------------------------------------------------------------------
        # encoder: e1 runs one step behind e0, phase-interleaved
        # ------------------------------------------------------------------
        m0_tiles = {}

        def mk_mix0(t, tr):
            mt = m0pool.tile([U, 2, R], F16, tag="m0", name="m0")
            mix_state(h0seq[:, t], mt, "act", 0, J, tr=tr)
            m0_tiles[t] = mt

        def e1_mk(tt):
            return cell_mk("e1", "e1_wx",
                           lambda m, tt=tt: h0seq[:, tt] if m == 0
                           else m0_tiles[tt][:, m - 1],
                           hsrc=(hz16 if tt == 0 else h1T), hmix=m1, hdst=h1T)

        def e0_mk(t):
            xc = xcats.pop(t)
            return cell_mk("e0", "e0_wx", lambda m, xc=xc: xc[:, m],
                           hsrc=(hz16 if t == 0 else h0seq[:, t - 1]),
                           hmix=(zmix if t == 0 else m0_tiles[t - 1]),
                           hdst=h0seq[:, t])

        dma_x(0)
        if t_enc > 1:
            dma_x(1)
        xstage_a(0)
        A = e0_mk(0)
        A.p_gates_h(); A.p_gates_x()
        for t in range(t_enc):
            if t + 2 < t_enc:
                dma_x(t + 2)
            if t + 1 < t_enc:
                xstage_a(t + 1)
            A.p_sig_rh()
            B = e1_mk(t - 1) if t >= 1 else None
            if B:
                B.p_gates_h(); B.p_gates_x()
            A.p_cand()
            if B:
                B.p_sig_rh()
            A.p_tail_h(0)
            if B:
                B.p_cand()
            mt = m0pool.tile([U, 2, R], F16, tag="m0", name="m0")
            m0_tiles[t] = mt
            mix_state(h0seq[:, t], mt, "act", 0, 3, tr="pe")
            A.p_tail_h(1)
            mix_state(h0seq[:, t], mt, "act", 3, 6, tr="pe")
            # software pipeline: next e0's h-side gate matmuls fill the
            # e1-tail / mix1 stall window (their inputs are ready here)
            if t + 1 < t_enc:
                An = e0_mk(t + 1)
                An.p_gates_h(); An.p_gates_x()
            if B:
                B.p_tail()
                mix_state(h1T, m1, "act", 0, J, tr="pe")
            if t + 1 < t_enc:
                A = An
        B = e1_mk(t_enc - 1)
        B.p_gates_h(); B.p_gates_x(); B.p_sig_rh(); B.p_cand(); B.p_tail()
        mix_state(h1T, m1, "act", 0, J, tr="pe")

        # ------------------------------------------------------------------
        # decoder: d0 x-side folded through WW on (h1, m1); proj output-only.
        # d1's h-side gate matmuls hoisted into d0's stall window.
        # ------------------------------------------------------------------
        def d0_mk(s):
            if s == 0:
                return cell_mk("d0", None, None,
                               hsrc=h0seq[:, t_enc - 1],
                               hmix=m0_tiles[t_enc - 1], hdst=hd0)
            return cell_mk("d0", "d0_ww",
                           lambda m: h1T if m == 0 else m1[:, m - 1],
                           hsrc=hd0, hmix=m0d, hdst=hd0)

        D0 = d0_mk(0)
        D0.p_gates_h()
        for s in range(t_dec):
            D1 = cell_mk("d1", "d1_wx",
                         lambda m: hd0 if m == 0 else m0d[:, m - 1],
                         hsrc=h1T, hmix=m1, hdst=h1T)
            D0.p_gates_x(); D0.p_sig_rh()
            D1.p_gates_h()
            D0.p_cand()
            D0.p_tail_h(0)
            mix_state(hd0, m0d, "dve", 0, 3, tr="pe")
            D0.p_tail_h(1)
            mix_state(hd0, m0d, "dve", 3, 6, tr="pe")
            D1.p_gates_x(); D1.p_sig_rh()
            # software pipeline: next d0's h-side gate matmuls (hd0_s and
            # m0d_s are ready) fill d1's cand/tail stall window
            if s + 1 < t_dec:
                D0n = d0_mk(s + 1)
                D0n.p_gates_h()
            D1.p_cand(); D1.p_tail()
            if s + 1 < t_dec:
                D0 = D0n
            # projection (output only, off critical path)
            pe_fence(h1T[:1, :1])
            for lo, sz, j0, j1 in HF:
                pp, k = scr_alloc([O, 304])
                mm(pp[:, :sz], w["wp"][:], h1T[:, lo:lo + sz])
                nc.scalar.activation(projT32[:, lo:lo + sz], pp[:, :sz],
                                     AF.Copy)
                scr_done(k, copy_act(tokT[:1, 6:7], projT32[:1, lo:lo + 1]),
                         tokT[:1, 6:7])
            nc.sync.dma_start(d["out"][s], projT32[:])
            mix_state(h1T, m1, "act", 0, J, tr="pe")



# --------------------------------------------------------------------------
# entry point
# --------------------------------------------------------------------------

def kernel(**inputs):
    arrs = _prep_host(inputs)
    nc = build_program(T, T)
    in_maps = []
    for core in range(NCORES):
        m = dict(arrs)
        m["xenc"] = _prep_xenc(inputs["encoder_inputs"], core)
        in_maps.append(m)
    res = run_bass_kernel_spmd(nc, in_maps, list(range(NCORES))).results
    outs = []
    for core in range(NCORES):
        o = np.asarray(res[core]["out"], np.float32)[:, :, :RV]   # [T, O, RV]
        o = o.reshape(T, O, BC, N).transpose(2, 0, 3, 1)          # [BC, T, N, O]
        outs.append(o)
    return np.ascontiguousarray(np.concatenate(outs, axis=0))

